# revision 1
# baseline (speedup 1.0000x reference)
"""Trainium2 Bass kernel for nn_CausalSelfAttention_52905407152466.

BitNet-style causal self-attention, distributed over 8 NeuronCores:
  - token-sharded QKV projections (512 tokens/core, full weights/core)
  - AllToAll #1 -> head-sharded attention (2 heads x B=2 per core)
  - AllToAll #2 -> token-sharded output projection

Numeric strategy: ternary weights are exact in fp16, so all projection
matmuls run in fp16 losslessly given fp16 activations. Attention runs in
fp16 (inputs ~2^-11 rounded, fp32 accumulation). The softmax skips the
max-subtraction (scores are bounded ~4) so exp folds into one activation
instruction per score group; the normalizer Z comes from an ones-column
appended to V. Causal masking = gpsimd affine_select on the exp output of
diagonal tiles. The final Wo projection uses the exact int8 path
(int8 x ternary in fp16 = exact integer accumulation in fp32).
"""

import numpy as np

import concourse.bacc as bacc
import concourse.mybir as mybir
import concourse.tile as tile
from concourse.bass_utils import run_bass_kernel_spmd
from concourse.masks import make_identity

F32 = mybir.dt.float32
F16 = mybir.dt.float16
I8 = mybir.dt.int8
AX = mybir.AxisListType
OP = mybir.AluOpType
ACTF = mybir.ActivationFunctionType

NCORES = 8
B, T, C = 2, 2048, 1024
H, D = 16, 64
BT = B * T                  # 4096 flat tokens
TPC = BT // NCORES          # 512 tokens per core
HPC = H // NCORES           # 2 heads per core
NT = TPC // 128             # 4 token tiles per core
NCT = C // 128              # 8 channel tiles
QB = 512                    # query block (free dim of score matmuls)
NQB = T // QB               # 4 query blocks per (b, h) instance
KT = 128                    # key tile (partition dim of scores)
ROPE_BASE = 10000.0

_CACHE = {}


def _host_tables(core):
    """Per-core RoPE tables in [128 = 2 interleaved heads x (32 lo | 32 hi), TPC] f16."""
    pos0 = (core * TPC) % T
    pos = np.arange(pos0, pos0 + TPC, dtype=np.float64)
    inv = 1.0 / (ROPE_BASE ** (np.arange(0, D, 2, dtype=np.float64) / D))
    ang = pos[None, :] * inv[:, None]              # [32, TPC]
    cos = np.cos(ang).astype(np.float32).astype(np.float16)
    sin = np.sin(ang).astype(np.float32).astype(np.float16)
    # rope as q*cos + (J q)*sin with J the half-swap sign matrix
    t1 = np.concatenate([cos, cos, cos, cos], axis=0)
    t2 = np.concatenate([sin, sin, sin, sin], axis=0)
    return t1.astype(np.float16), t2.astype(np.float16)


def _host_jt():
    i32 = np.eye(32, dtype=np.float16)
    z = np.zeros((32, 32), np.float16)
    j64 = np.block([[z, -i32], [i32, z]])     # J: Jq[0:32] = -q[32:64]; Jq[32:64] = q[0:32]
    jt = np.block([[j64.T, np.zeros((64, 64), np.float16)],
                   [np.zeros((64, 64), np.float16), j64.T]])
    return jt.astype(np.float16)


def build_program():
    nc = bacc.Bacc("TRN2", target_bir_lowering=False, debug=False,
                   num_devices=NCORES)
    io = {}

    def inp(name, shape, dtype=F32):
        io[name] = nc.declare_dram_parameter(name, list(shape), dtype, isOutput=False)
        return io[name]

    def outp(name, shape, dtype=F32):
        io[name] = nc.declare_dram_parameter(name, list(shape), dtype, isOutput=True)
        return io[name]

    x_d = inp("x_slice", (TPC, C))
    w_d = {n: inp(n + "T", (C, C)) for n in ("Wq", "Wk", "Wv", "Wo")}
    t1_d = inp("ropeT1", (128, TPC), F16)
    t2_d = inp("ropeT2", (128, TPC), F16)
    jt_d = inp("ropeJT", (128, 128), F16)
    out_d = outp("out_slice", (TPC, C))

    import os
    skip_coll = os.environ.get("SKIP_COLL", "0") == "1"
    # layout per shard: q [128, TPC], k [128, TPC], v [128, NT, 130]
    with tile.TileContext(nc) as tc:
        with tc.tile_pool(name="dram", bufs=1, space="DRAM") as dram:
            a2a1_in = dram.tile([NCORES, 2 * 128 * TPC], F16)
            a2a1_out = dram.tile([NCORES, 2 * 128 * TPC], F16)
            a2av_in = dram.tile([NCORES, 128 * NT * 130], F16)
            a2av_out = dram.tile([NCORES, 128 * NT * 130], F16)
            a2a2_in = dram.tile([NCORES, 128 * NT * 2 * D], F16)
            a2a2_out = dram.tile([NCORES, 128 * NT * 2 * D], F16)

            _build_body(nc, tc, io, a2a1_in, a2a1_out, a2av_in, a2av_out,
                        a2a2_in, a2a2_out, skip_coll=skip_coll)
    nc.compile()
    return nc


def _build_body(nc, tc, io, a2a1_in, a2a1_out, a2av_in, a2av_out,
                a2a2_in, a2a2_out, skip_coll=False):
    from contextlib import ExitStack
    es = ExitStack()
    ident_pool = es.enter_context(tc.tile_pool(name="const", bufs=1))
    sb = es.enter_context(tc.tile_pool(name="sb", bufs=1))
    wl = es.enter_context(tc.tile_pool(name="wl", bufs=2))
    esp = ExitStack()
    ps = esp.enter_context(tc.tile_pool(name="mmps", bufs=3, space="PSUM"))
    psy = esp.enter_context(tc.tile_pool(name="psy", bufs=1, space="PSUM"))

    # ---------------- constants -------------------------------------------
    ident = ident_pool.tile([128, 128], F16)
    make_identity(nc, ident[:])
    t1 = ident_pool.tile([128, TPC], F16)
    t2 = ident_pool.tile([128, TPC], F16)
    nc.sync.dma_start(t1[:], io["ropeT1"][:])
    nc.sync.dma_start(t2[:], io["ropeT2"][:])
    jt = ident_pool.tile([128, 128], F16)
    nc.sync.dma_start(jt[:], io["ropeJT"][:])
    ones_col = ident_pool.tile([128, 1], F16)
    nc.gpsimd.memset(ones_col[:], 1.0)

    # ---------------- P0: x load + act_quant + transpose ------------------
    xsb = sb.tile([128, NT, C], F32)
    nc.sync.dma_start(xsb[:], io["x_slice"].rearrange("(n p) c -> p n c", p=128))
    xq16 = sb.tile([128, NT, C], F16)
    for tt in range(NT):
        mx = sb.tile([128, 1], F32, tag="mx")
        nc.vector.tensor_reduce(mx[:], xsb[:, tt], axis=AX.X, op=OP.max,
                                apply_absolute_value=True)
        sc = sb.tile([128, 1], F32, tag="sc")   # 1/st = clip(mx)/127
        nc.vector.tensor_scalar(sc[:], mx[:], 1e-5, 1.0 / 127.0,
                                op0=OP.max, op1=OP.mult)
        st = sb.tile([128, 1], F32, tag="st")   # 127/clip(mx)
        nc.vector.reciprocal(st[:], sc[:])
        xq8 = sb.tile([128, C], I8, tag="xq8")
        nc.vector.tensor_scalar(xq8[:], xsb[:, tt], st[:], None, op0=OP.mult)
        nc.vector.tensor_scalar(xq16[:, tt], xq8[:], sc[:], None, op0=OP.mult)
    # transpose -> xqT [c, t] tiles (PE transpose, psum bounce)
    xqT = sb.tile([128, NCT, TPC], F16)
    for ct in range(NCT):
        for tt in range(NT):
            trx = psy.tile([128, 128], F16, tag="trx")
            nc.tensor.transpose(trx[:], xq16[:, tt, 128 * ct:128 * (ct + 1)], ident[:])
            nc.scalar.activation(xqT[:, ct, 128 * tt:128 * (tt + 1)], trx[:], ACTF.Copy)

    # ---------------- weights helper ---------------------------------------
    wT = {}
    swcol = {}
    ones128 = ident_pool.tile([1, 128], F32)
    nc.gpsimd.memset(ones128[:], 1.0)
    onescol32 = ident_pool.tile([128, 1], F32)
    nc.gpsimd.memset(onescol32[:], 1.0)

    def prep_weight(wn, tern_eng=None, act_asums=False):
        tern_eng = tern_eng or nc.vector
        wsb = wl.tile([128, NCT, C], F32, tag="wload", name=f"wload_{wn}")
        nc.sync.dma_start(wsb[:], io[wn + "T"].rearrange("(n p) c -> p n c", p=128))
        asums = sb.tile([128, NCT], F32, tag="asums", name=f"asums_{wn}")
        for ot in range(NCT):
            if act_asums:
                junk = sb.tile([128, C], F16, tag="junk", name=f"junk_{wn}{ot}",
                               bufs=2)
                nc.scalar.activation(junk[:], wsb[:, ot], ACTF.Abs,
                                     accum_out=asums[:, ot:ot + 1])
            else:
                nc.vector.tensor_reduce(asums[:, ot:ot + 1], wsb[:, ot], axis=AX.X,
                                        op=OP.add, apply_absolute_value=True)
        atot = sb.tile([128, 1], F32, tag="atot", name=f"atot_{wn}")
        nc.vector.tensor_reduce(atot[:], asums[:], axis=AX.X, op=OP.add)
        sw_ps = psy.tile([1, 1], F32, tag="swps", name=f"swps_{wn}")
        nc.tensor.matmul(sw_ps[:], onescol32[:], atot[:], start=True, stop=True)
        sw = sb.tile([1, 1], F32, tag="sw", name=f"sw_{wn}")
        nc.vector.tensor_scalar(sw[:], sw_ps[:], 1.0 / (C * C), 1e-5,
                                op0=OP.mult, op1=OP.max)
        swb_ps = psy.tile([128, 1], F32, tag="swbps", name=f"swbps_{wn}")
        nc.tensor.matmul(swb_ps[:], ones128[:], sw[:], start=True, stop=True)
        swc = sb.tile([128, 1], F32, tag=f"swc_{wn}", name=f"swc_{wn}")
        nc.vector.tensor_copy(swc[:], swb_ps[:])
        swcol[wn] = swc
        inv_s = sb.tile([128, 1], F32, tag="inv_s", name=f"invs_{wn}")
        nc.vector.reciprocal(inv_s[:], swc[:])
        wtag = "wTs_0" if wn in ("Wq", "Wv") else "wTs_1"
        wTt = sb.tile([128, NCT, C], F16, tag=wtag, name=f"wT_{wn}")
        for ot in range(NCT):
            w8 = sb.tile([128, C], I8, tag="w8", name=f"w8_{wn}{ot}", bufs=2)
            tern_eng.tensor_scalar(w8[:], wsb[:, ot], inv_s[:], None, op0=OP.mult)
            tern_eng.tensor_scalar(wTt[:, ot], w8[:], 1, -1,
                                   op0=OP.min, op1=OP.max)
        wT[wn] = wTt

    def proj_qk(name, dst):
        for ot in range(NCT):
            mm_ps = ps.tile([128, TPC], F32, tag="mm512", name=f"mmps_{name}{ot}")
            for ct in range(NCT):
                nc.tensor.matmul(mm_ps[:], wT[name][:, ct, 128 * ot:128 * (ot + 1)],
                                 xqT[:, ct], start=(ct == 0), stop=(ct == NCT - 1))
            raw = sb.tile([128, TPC], F16, tag="qkraw", name=f"raw_{name}{ot}")
            nc.vector.tensor_copy(raw[:], mm_ps[:])
            jq_ps = ps.tile([128, TPC], F32, tag="mm512", name=f"jq_{name}{ot}")
            nc.tensor.matmul(jq_ps[:], jt[:], raw[:], start=True, stop=True)
            p1 = sb.tile([128, TPC], F16, tag="ropep1", name=f"p1_{name}{ot}")
            p2 = sb.tile([128, TPC], F16, tag="ropep2", name=f"p2_{name}{ot}")
            nc.vector.tensor_tensor(p1[:], raw[:], t1[:], op=OP.mult)
            nc.vector.tensor_tensor(p2[:], jq_ps[:], t2[:], op=OP.mult)
            nc.vector.tensor_tensor(dst[:, ot], p1[:], p2[:], op=OP.add)

    QSZ = 128 * TPC
    VSZ = 128 * NT * 130

    # v pipeline -> atoa-v
    prep_weight("Wv")
    v_sb = sb.tile([128, NT, H, 65], F16)
    nc.gpsimd.memset(v_sb[:], 1.0)
    for tt in range(NT):
        for ob in range(2):
            mm_ps = ps.tile([128, 512], F32, tag="mm512", name=f"vps_{tt}{ob}")
            for ct in range(NCT):
                nc.tensor.matmul(mm_ps[:], xqT[:, ct, 128 * tt:128 * (tt + 1)],
                                 wT["Wv"][:, ct, 512 * ob:512 * (ob + 1)],
                                 start=(ct == 0), stop=(ct == NCT - 1))
            nc.scalar.activation(
                v_sb[:, tt, 8 * ob:8 * (ob + 1), 0:64], mm_ps[:],
                ACTF.Copy, scale=swcol["Wv"][:])
    for dst in range(NCORES):
        eng = nc.sync if dst % 2 == 0 else nc.scalar
        eng.dma_start(
            a2av_in[dst].rearrange("(p n v) -> p n v", p=128, n=NT),
            v_sb[:, :, 2 * dst:2 * dst + 2, :].rearrange("p n h v -> p n (h v)"))
    if skip_coll:
        nc.sync.dma_start(a2av_out[:], a2av_in[:])
    else:
        nc.gpsimd.collective_compute(
            "AllToAll", OP.bypass, replica_groups=[list(range(NCORES))],
            ins=[a2av_in.opt()], outs=[a2av_out.opt()])

    va = sb.tile([128, BT // 128, 2, 65], F16, tag="v_sb")   # reuse v_sb slot
    for s in range(NCORES):
        eng = nc.sync if s % 2 == 0 else nc.scalar
        eng.dma_start(
            va[:, NT * s:NT * (s + 1)].rearrange("p n h v -> p n (h v)"),
            a2av_out[s].rearrange("(p n v) -> p n v", p=128, n=NT))

    # q/k pipeline -> atoa-qk as early as possible
    qTr = sb.tile([128, NCT, TPC], F16)
    kTr = sb.tile([128, NCT, TPC], F16)
    prep_weight("Wq")
    proj_qk("Wq", qTr)
    nc.sync.dma_start(a2a1_in[:, 0:QSZ].rearrange("d (p t) -> p d t", p=128), qTr[:])
    prep_weight("Wk")
    proj_qk("Wk", kTr)
    nc.sync.dma_start(a2a1_in[:, QSZ:2 * QSZ].rearrange("d (p t) -> p d t", p=128),
                      kTr[:])
    if skip_coll:
        nc.sync.dma_start(a2a1_out[:], a2a1_in[:])
    else:
        nc.gpsimd.collective_compute(
            "AllToAll", OP.bypass, replica_groups=[list(range(NCORES))],
            ins=[a2a1_in.opt()], outs=[a2a1_out.opt()])

    qTa = sb.tile([128, BT], F16, tag="qTr")     # reuse qTr slot (dead after send)
    kTa = sb.tile([128, BT], F16, tag="kTr")
    nc.scalar.dma_start(qTa[:].rearrange("p (s t) -> p s t", s=NCORES),
                        a2a1_out[:, 0:QSZ].rearrange("s (p t) -> p s t", p=128))
    nc.scalar.dma_start(kTa[:].rearrange("p (s t) -> p s t", s=NCORES),
                        a2a1_out[:, QSZ:2 * QSZ].rearrange("s (p t) -> p s t", p=128))

    # exp scale column: swq*swk/8 -> [128,1] f32
    expsc = sb.tile([128, 1], F32)
    nc.vector.tensor_tensor(expsc[:], swcol["Wq"][:], swcol["Wk"][:], op=OP.mult)
    nc.vector.tensor_scalar(expsc[:], expsc[:], 1.0 / np.sqrt(np.float64(D)), None,
                            op0=OP.mult)

    # Wo prep overlaps the collectives / attention start
    prep_weight("Wo", tern_eng=nc.gpsimd)

    # ---------------- P4: attention ----------------------------------------
    # per (head, batch, q-block): scores (transposed) -> exp -> mask -> AV
    esp.close()
    y_sb = sb.tile([128, BT // 128, 2, D], F16)   # [qt-part, qt-tile, head, d]
    exp_sb_pool = es.enter_context(tc.tile_pool(name="expp", bufs=4))
    esp = ExitStack()
    score_ps_pool = esp.enter_context(tc.tile_pool(name="scps", bufs=3, space="PSUM"))
    yaug_ps_pool = esp.enter_context(tc.tile_pool(name="yaug", bufs=1, space="PSUM"))
    tr_ps_pool = esp.enter_context(tc.tile_pool(name="trps", bufs=1, space="PSUM"))
    KG = 2          # k-tiles per exp group (psum banks per score group)
    for b in range(B):
        base = b * T
        for jb in range(NQB):
            qs = base + QB * jb           # q-block col offset
            for h in range(HPC):
                yaug = yaug_ps_pool.tile([65, QB], F32, tag="yaug")
                nkt = (jb + 1) * (QB // KT)       # causal k-tiles for this block
                ngrp = nkt // KG
                # diagonal (masked) groups first: their affine_select latency
                # hides behind later groups' score/exp stream
                kg_order = list(reversed(range(ngrp)))
                for kgi, kg in enumerate(kg_order):
                    sgrp = score_ps_pool.tile([128, KG * QB], F32, tag="sgrp")
                    for m in range(KG):
                        kt_i = kg * KG + m
                        ks = base + KT * kt_i
                        nc.tensor.matmul(
                            sgrp[:, QB * m:QB * (m + 1)],
                            kTa[64 * h:64 * (h + 1), ks:ks + KT],
                            qTa[64 * h:64 * (h + 1), qs:qs + QB],
                            start=True, stop=True,
                            tile_position=(64 * h, 0))
                    egrp = exp_sb_pool.tile([128, KG * QB], F16, tag=f"egrp{h}")
                    nc.scalar.activation(egrp[:], sgrp[:], ACTF.Exp, scale=expsc[:])
                    for m in range(KG):
                        kt_i = kg * KG + m
                        mbase = QB * jb - KT * kt_i
                        if mbase < 127:   # diagonal tile: causal mask needed
                            nc.gpsimd.affine_select(
                                out=egrp[:, QB * m:QB * (m + 1)],
                                in_=egrp[:, QB * m:QB * (m + 1)],
                                compare_op=OP.is_ge, fill=0.0,
                                base=mbase, pattern=[[1, QB]],
                                channel_multiplier=-1)
                    for m in range(KG):
                        kt_i = kg * KG + m
                        gt = base // 128 + kt_i
                        nc.tensor.matmul(yaug[:], va[:, gt, h, :],
                                         egrp[:, QB * m:QB * (m + 1)],
                                         start=(kgi == 0 and m == 0),
                                         stop=(kgi == ngrp - 1 and m == KG - 1))
                # epilogue: copy, transpose 128-chunks, normalize
                yaug16 = exp_sb_pool.tile([65, QB], F16, tag=f"yaug16_{h}", bufs=2)
                nc.vector.tensor_copy(yaug16[:], yaug[:])
                for ch in range(QB // 128):
                    trp = tr_ps_pool.tile([128, 65], F16, tag="trp")
                    nc.tensor.transpose(trp[:], yaug16[:, 128 * ch:128 * (ch + 1)],
                                        ident[0:65, 0:65])
                    rec = exp_sb_pool.tile([128, 1], F32, tag=f"rec{h}", bufs=2)
                    nc.vector.reciprocal(rec[:], trp[:, 64:65])
                    nc.vector.tensor_scalar(
                        y_sb[:, (qs + 128 * ch) // 128, h, :], trp[:, 0:64],
                        rec[:], None, op0=OP.mult)

    # ---------------- P5: AllToAll #2 --------------------------------------
    YSZ = 128 * NT * 2 * D
    for dst in range(NCORES):
        eng = nc.sync if dst % 2 == 0 else nc.scalar
        eng.dma_start(
            a2a2_in[dst].rearrange("(p n c) -> p n c", p=128, n=NT),
            y_sb[:, NT * dst:NT * (dst + 1)].rearrange("p n h dd -> p n (h dd)"))
    if skip_coll:
        nc.sync.dma_start(a2a2_out[:], a2a2_in[:])
    else:
        nc.gpsimd.collective_compute(
            "AllToAll", OP.bypass, replica_groups=[list(range(NCORES))],
            ins=[a2a2_in.opt()], outs=[a2a2_out.opt()])
    yfull = sb.tile([128, NT, C], F16, tag="xq16")   # [t-part, t-tile, channels]
    for s in range(NCORES):
        eng = nc.sync if s % 2 == 0 else nc.scalar
        eng.dma_start(
            yfull[:, :, 128 * s:128 * (s + 1)],
            a2a2_out[s].rearrange("(p n c) -> p n c", p=128, n=NT))

    # act_quant(y) exact int8 + transpose
    esp.close()
    esp = ExitStack()
    ps = esp.enter_context(tc.tile_pool(name="ops", bufs=4, space="PSUM"))
    yq16 = sb.tile([128, NT, C], F16)
    osc = {}
    for tt in range(NT):
        mxy = sb.tile([128, 1], F32, tag="mxy")
        nc.vector.tensor_reduce(mxy[:], yfull[:, tt], axis=AX.X, op=OP.max,
                                apply_absolute_value=True)
        scy = sb.tile([128, 1], F32, tag=f"scy{tt}")
        nc.vector.tensor_scalar(scy[:], mxy[:], 1e-5, 1.0 / 127.0,
                                op0=OP.max, op1=OP.mult)
        sty = sb.tile([128, 1], F32, tag="sty")
        nc.vector.reciprocal(sty[:], scy[:])
        yq8 = sb.tile([128, C], I8, tag="yq8")
        nc.vector.tensor_scalar(yq8[:], yfull[:, tt], sty[:], None, op0=OP.mult)
        nc.vector.tensor_copy(yq16[:, tt], yq8[:])
        # output scale column: swo * scy
        oscc = sb.tile([128, 1], F32, tag=f"oscc{tt}")
        nc.vector.tensor_tensor(oscc[:], scy[:], swcol["Wo"][:], op=OP.mult)
        osc[tt] = oscc
    yqT = sb.tile([128, NCT, TPC], F16)
    for ct in range(NCT):
        for tt in range(NT):
            trx = ps.tile([128, 128], F16, tag="trx")
            nc.tensor.transpose(trx[:], yq16[:, tt, 128 * ct:128 * (ct + 1)], ident[:])
            nc.vector.tensor_copy(yqT[:, ct, 128 * tt:128 * (tt + 1)], trx[:])

    # ---------------- P6: Wo projection ------------------------------------
    out_sb = sb.tile([128, NT, C], F32, tag="xsb")
    for tt in range(NT):
        for ob in range(2):
            mm_ps = ps.tile([128, 512], F32, tag="mm512")
            for ct in range(NCT):
                nc.tensor.matmul(mm_ps[:], yqT[:, ct, 128 * tt:128 * (tt + 1)],
                                 wT["Wo"][:, ct, 512 * ob:512 * (ob + 1)],
                                 start=(ct == 0), stop=(ct == NCT - 1))
            nc.scalar.activation(out_sb[:, tt, 512 * ob:512 * (ob + 1)], mm_ps[:],
                                 ACTF.Copy, scale=osc[tt][:])
            nc.sync.dma_start(
                io["out_slice"].rearrange("(n p) c -> p n c", p=128)
                [:, tt, 512 * ob:512 * (ob + 1)],
                out_sb[:, tt, 512 * ob:512 * (ob + 1)])
    esp.close()
    es.close()


def kernel(x, Wq, Wk, Wv, Wo, _trace=False):
    x = np.ascontiguousarray(x, dtype=np.float32)
    if "nc" not in _CACHE:
        _CACHE["nc"] = build_program()
    nc = _CACHE["nc"]
    xf = x.reshape(BT, C)
    wqT = np.ascontiguousarray(np.asarray(Wq, np.float32).T)
    wkT = np.ascontiguousarray(np.asarray(Wk, np.float32).T)
    wvT = np.ascontiguousarray(np.asarray(Wv, np.float32).T)
    woT = np.ascontiguousarray(np.asarray(Wo, np.float32).T)
    in_maps = []
    for c in range(NCORES):
        t1, t2 = _host_tables(c)
        in_maps.append({
            "x_slice": np.ascontiguousarray(xf[TPC * c:TPC * (c + 1)]),
            "WqT": wqT, "WkT": wkT, "WvT": wvT, "WoT": woT,
            "ropeT1": t1, "ropeT2": t2, "ropeJT": _host_jt(),
        })
    res = run_bass_kernel_spmd(nc, in_maps, list(range(NCORES)), trace=_trace)
    out = np.concatenate([res.results[c]["out_slice"] for c in range(NCORES)], axis=0)
    out = out.reshape(B, T, C).astype(np.float32)
    if _trace:
        return out, res
    return out



# revision 13
# speedup vs baseline: 1.1902x; 1.1902x over previous
"""Trainium2 Bass kernel for nn_CausalSelfAttention_52905407152466.

BitNet-style causal self-attention, distributed over 8 NeuronCores with an
instance-parallel (batch x head-group) sharding that needs NO collective
before attention:

  - core c owns batch b=c//4 and heads {4j..4j+3} with j=c%4.  It receives
    its batch's x (transposed, fp16) and the column slices of Wq/Wk/Wv for
    its heads, computes q,k,v for all 2048 tokens of its batch locally, and
    runs causal attention for its 4 heads.
  - the ternary weight scales (mean|W|) need the full matrices; each core
    reduces a 1/8 row shard of each W and a 128-byte AllGather combines the
    partial sums (fully overlapped with the x pipeline).
  - one AllToAll reshards y from (batch,head)-sharded to token-sharded for
    the output projection, split into two 0.5MB halves so the first half
    transfers while the second half of attention still computes.

Numerics: act_quant int8 values split EXACTLY into two fp8e4m3 operands
(a=fp8(v), b=v-a with |b|<=4; both exact), and ternary weights are exact in
fp8, so every projection runs as DoubleRow fp8 matmuls (2x fp16 throughput)
while reproducing the reference int8xternary products exactly (fp32 psum).
Per-token activation-quant scales are folded into the rope tables (q,k),
the exp scale (sw_q*sw_k/sqrt(D)), the v psum copy (sc*sw_v) and the output
copy (scy*sw_o).  Attention itself runs in fp16 with the ones-column-in-V
normalizer and gpsimd affine_select causal masks, like the reference
token-sharded kernel this replaces.
"""

import numpy as np

import concourse.bacc as bacc
import concourse.mybir as mybir
import concourse.tile as tile
from concourse.bass_utils import run_bass_kernel_spmd
from concourse.masks import make_identity

F32 = mybir.dt.float32
F16 = mybir.dt.float16
F8 = mybir.dt.float8e4
I8 = mybir.dt.int8
AX = mybir.AxisListType
OP = mybir.AluOpType
ACTF = mybir.ActivationFunctionType
DR = mybir.MatmulPerfMode.DoubleRow

NCORES = 8
B, T, C = 2, 2048, 1024
H, D = 16, 64
HPC = 4                     # heads per core
HD = HPC * D                # 256 projection channels per core
NCT = C // 128              # 8 channel tiles
NCP = NCT // 2              # 4 channel-tile pairs (DoubleRow)
NTT = T // 128              # 16 token tiles per batch
QB = 512                    # query block
NQB = T // QB               # 4
KT = 128                    # key tile
KG = 2                      # key tiles per exp group
OT = 512                    # owned output tokens per core (256 per batch)
OTT = OT // 128             # 4
CH = 512                    # q/k projection token chunk
NCH = T // CH               # 4
ASZ = 128 * 2 * 2 * D       # a2a slot elems: 128p x 2 tiles x 2 heads x 64
ROPE_BASE = 10000.0

_CACHE = {}


def _host_tables():
    pos = np.arange(T, dtype=np.float64)
    inv = 1.0 / (ROPE_BASE ** (np.arange(0, D, 2, dtype=np.float64) / D))
    ang = pos[None, :] * inv[:, None]              # [32, T]
    cos = np.cos(ang).astype(np.float32).astype(np.float16)
    sin = np.sin(ang).astype(np.float32).astype(np.float16)
    t1 = np.concatenate([cos, cos, cos, cos], axis=0)
    t2 = np.concatenate([sin, sin, sin, sin], axis=0)
    return t1.astype(np.float16), t2.astype(np.float16)


def _host_jt():
    i32 = np.eye(32, dtype=np.float16)
    z = np.zeros((32, 32), np.float16)
    j64 = np.block([[z, -i32], [i32, z]])     # J: Jq[0:32] = -q[32:64]; Jq[32:64] = q[0:32]
    jt = np.block([[j64.T, np.zeros((64, 64), np.float16)],
                   [np.zeros((64, 64), np.float16), j64.T]])
    return jt.astype(np.float16)


def build_program():
    nc = bacc.Bacc("TRN2", target_bir_lowering=False, debug=False,
                   num_devices=NCORES)
    io = {}

    def inp(name, shape, dtype=F32):
        io[name] = nc.declare_dram_parameter(name, list(shape), dtype, isOutput=False)
        return io[name]

    def outp(name, shape, dtype=F32):
        io[name] = nc.declare_dram_parameter(name, list(shape), dtype, isOutput=True)
        return io[name]

    inp("xT16", (C, T), F16)
    inp("Wshards", (128, 4, C))
    for n in ("Wq", "Wk", "Wv"):
        inp(n + "Ts", (C, HD))
    inp("WoT", (C, C))
    inp("ropeT1", (128, T), F16)
    inp("ropeT2", (128, T), F16)
    inp("ropeJT", (128, 128), F16)
    outp("out_slice", (OT, C))

    with tile.TileContext(nc) as tc:
        with tc.tile_pool(name="dram", bufs=1, space="DRAM") as dram:
            ag_in = dram.tile([1, 4], F32)
            ag_out = dram.tile([8, 4], F32)
            a2aA_in = dram.tile([NCORES, ASZ], F16)
            a2aA_out = dram.tile([NCORES, ASZ], F16)
            a2aB_in = dram.tile([NCORES, ASZ], F16)
            a2aB_out = dram.tile([NCORES, ASZ], F16)
            _build_body(nc, tc, io, ag_in, ag_out,
                        a2aA_in, a2aA_out, a2aB_in, a2aB_out)
    nc.compile()
    return nc


def _build_body(nc, tc, io, ag_in, ag_out, a2aA_in, a2aA_out, a2aB_in, a2aB_out):
    from contextlib import ExitStack
    es = ExitStack()
    const = es.enter_context(tc.tile_pool(name="const", bufs=1))
    sb = es.enter_context(tc.tile_pool(name="sb", bufs=1))
    front = ExitStack()
    fr = front.enter_context(tc.tile_pool(name="fr", bufs=1))
    wl = front.enter_context(tc.tile_pool(name="wl", bufs=1))
    psA = ExitStack()
    ppa = psA.enter_context(tc.tile_pool(name="ppa", bufs=1, space="PSUM"))

    # ---------------- constants -------------------------------------------
    id16 = const.tile([128, 128], F16)
    make_identity(nc, id16[:])
    id8 = const.tile([128, 128], F8)
    make_identity(nc, id8[:])
    idf = const.tile([128, 128], F32)
    make_identity(nc, idf[:])
    t1 = const.tile([128, T], F16)
    t2 = const.tile([128, T], F16)
    jt = const.tile([128, 128], F16)
    nc.scalar.dma_start(t1[:], io["ropeT1"][:])
    nc.scalar.dma_start(t2[:], io["ropeT2"][:])
    nc.scalar.dma_start(jt[:], io["ropeJT"][:])
    ones_col = const.tile([128, 1], F32)
    nc.gpsimd.memset(ones_col[:], 1.0)
    ones_row = const.tile([1, 128], F32)
    nc.gpsimd.memset(ones_row[:], 1.0)
    ones_row16 = const.tile([1, 128], F16)
    nc.gpsimd.memset(ones_row16[:], 1.0)

    # ---------------- P0: weight-scale shards + AllGather ------------------
    asums = fr.tile([128, 4], F32)
    for w in range(4):
        wsh = fr.tile([128, 1, C], F32, tag="wsh", bufs=2, name=f"wsh_{w}")
        nc.sync.dma_start(wsh[:], io["Wshards"][:, w:w + 1])
        nc.vector.tensor_reduce(asums[:, w:w + 1], wsh[:, 0], axis=AX.X, op=OP.add,
                                apply_absolute_value=True)
    part_ps = ppa.tile([1, 4], F32, tag="pp_small")
    nc.tensor.matmul(part_ps[:], ones_col[:], asums[:], start=True, stop=True)
    part_sb = fr.tile([1, 4], F32)
    nc.vector.tensor_copy(part_sb[:], part_ps[:])
    nc.scalar.dma_start(ag_in[:], part_sb[:])
    nc.gpsimd.collective_compute(
        "AllGather", OP.bypass, replica_groups=[list(range(NCORES))],
        ins=[ag_in.opt()], outs=[ag_out.opt()])
    gath = fr.tile([8, 4], F32)
    nc.scalar.dma_start(gath[:], ag_out[:])
    tot_ps = ppa.tile([1, 4], F32, tag="pp_small", name="tot_ps")
    nc.tensor.matmul(tot_ps[:], ones_col[0:8], gath[:], start=True, stop=True)
    sw_row = fr.tile([1, 4], F32)
    nc.vector.tensor_scalar(sw_row[:], tot_ps[:], 1.0 / (C * C), 1e-5,
                            op0=OP.mult, op1=OP.max)
    swc = {}
    inv_s = {}
    for wi, wn in enumerate(("Wq", "Wk", "Wv", "Wo")):
        swb_ps = ppa.tile([128, 1], F32, tag="pp_small2", name=f"swb_{wn}")
        nc.tensor.matmul(swb_ps[:], ones_row[:], sw_row[:, wi:wi + 1],
                         start=True, stop=True)
        c_ = sb.tile([128, 1], F32, name=f"swc_{wn}")
        nc.vector.tensor_copy(c_[:], swb_ps[:])
        swc[wn] = c_
        iv = sb.tile([128, 1], F32, name=f"invs_{wn}")
        nc.vector.reciprocal(iv[:], c_[:])
        inv_s[wn] = iv

    # ---------------- P1: x load + per-token absmax ------------------------
    xTr = io["xT16"].rearrange("(ct p) t -> p ct t", p=128)
    accA = fr.tile([128, T], F16)
    accB = fr.tile([128, T], F16)
    for ct in range(NCT):
        xc = fr.tile([128, T], F16, tag="xs1", bufs=2, name=f"xs1_{ct}")
        eng = nc.sync if ct % 2 == 0 else nc.scalar
        eng.dma_start(xc[:], xTr[:, ct])
        acc, veng = ((accA, nc.vector), (accB, nc.gpsimd))[ct % 2]
        if ct < 2:
            veng.tensor_scalar(acc[:], xc[:], 0.0, None, op0=OP.abs_max)
        else:
            veng.tensor_tensor(acc[:], acc[:], xc[:], op=OP.abs_max)
    acc = accA
    nc.vector.tensor_tensor(acc[:], accA[:], accB[:], op=OP.abs_max)

    # per-token channel max is a partition (C-axis) reduction on gpsimd
    mx_row = fr.tile([1, T], F32)
    nc.gpsimd.tensor_reduce(mx_row[:], acc[:], axis=AX.C, op=OP.max)
    sc_row = mx_row   # in-place: mx_row becomes the dequant scale row
    nc.vector.tensor_scalar(sc_row[:], mx_row[:], 1e-5, 1.0 / 127.0,
                            op0=OP.max, op1=OP.mult)
    st_row = fr.tile([1, T], F32)
    nc.vector.reciprocal(st_row[:], sc_row[:])
    scrow16 = fr.tile([1, T], F16)
    nc.vector.tensor_copy(scrow16[:], sc_row[:])

    # broadcast sc (f16, dequant) then st (f32, grid-exact) along partitions
    scb_ps = ppa.tile([128, T], F32, tag="bigps")
    nc.tensor.matmul(scb_ps[:], ones_row16[:], scrow16[:], start=True, stop=True)
    scb16 = fr.tile([128, T], F16)
    nc.gpsimd.tensor_copy(scb16[:], scb_ps[:])
    t1s = sb.tile([128, T], F16)
    t2s = sb.tile([128, T], F16)
    nc.vector.tensor_tensor(t1s[:], t1[:], scb16[:], op=OP.mult)
    nc.gpsimd.tensor_tensor(t2s[:], t2[:], scb16[:], op=OP.mult)
    stb = ppa.tile([128, T], F32, tag="bigps")   # psum-resident through quant
    nc.tensor.matmul(stb[:], ones_row[:], st_row[:], start=True, stop=True)

    # token-major v-copy scale sv[p, tt] = sc(token 128*tt+p) * sw_v, via
    # transposing the (all-rows-identical) broadcast back per token tile
    sv = fr.tile([128, NTT, 1], F32)
    for tt in range(NTT):
        trs = ppa.tile([128, 128], F16, tag="trs", bufs=1, name=f"trs_{tt}")
        nc.tensor.transpose(trs[:], scb16[:, 128 * tt:128 * (tt + 1)], id16[:])
        nc.vector.tensor_scalar(sv[:, tt], trs[:, 0:1], swc["Wv"][:], None,
                                op0=OP.mult)

    # quantize: xq8 int8 grid == fp8 a + fp8 b exactly  (x re-streamed)
    a8 = fr.tile([128, NCT, T], F8)
    b8 = fr.tile([128, NCT, T], F8)
    for ct in range(NCT):
        xc = fr.tile([128, T], F16, tag="xs2", bufs=2, name=f"xs2_{ct}")
        eng = nc.sync if ct % 2 == 0 else nc.scalar
        eng.dma_start(xc[:], xTr[:, ct])
        xq8 = fr.tile([128, T], I8, tag="xq8", bufs=2, name=f"xq8_{ct}")
        nc.vector.tensor_tensor(xq8[:], xc[:], stb[:], op=OP.mult)
        nc.gpsimd.tensor_copy(a8[:, ct], xq8[:])
        eng2 = nc.vector if ct % 2 == 0 else nc.gpsimd
        eng2.tensor_tensor(b8[:, ct], xq8[:], a8[:, ct], op=OP.subtract)

    # ---------------- P2: weight slices, ternarize to fp8 ------------------
    w8 = {}
    for i, wn in enumerate(("Wq", "Wk", "Wv")):
        wst = wl.tile([128, NCT, HD], F32, tag="wst", bufs=2, name=f"wst_{wn}")
        nc.sync.dma_start(wst[:], io[wn + "Ts"].rearrange("(ct p) o -> p ct o", p=128))
        wi8 = fr.tile([128, NCT, HD], I8, tag="wi8", bufs=2, name=f"wi8_{wn}")
        e1, e2 = (nc.vector, nc.gpsimd) if i % 2 == 0 else (nc.gpsimd, nc.vector)
        e1.tensor_scalar(wi8[:], wst[:], inv_s[wn][:], None, op0=OP.mult)
        wf = sb.tile([128, NCT, HD], F8, name=f"w8_{wn}")
        e2.tensor_scalar(wf[:], wi8[:], 1, -1, op0=OP.min, op1=OP.max)
        w8[wn] = wf
    wo8 = sb.tile([128, NCT, C], F8)
    woR = io["WoT"].rearrange("(ct p) o -> p ct o", p=128)
    for chunk in range(4):
        wst = wl.tile([128, 2, C], F32, tag="woc", bufs=2, name=f"woc_{chunk}")
        nc.sync.dma_start(wst[:], woR[:, 2 * chunk:2 * chunk + 2])
        wi8 = fr.tile([128, 2, C], I8, tag="woi", bufs=2, name=f"woi_{chunk}")
        e1, e2 = (nc.vector, nc.gpsimd) if chunk % 2 == 0 else (nc.gpsimd, nc.vector)
        e1.tensor_scalar(wi8[:], wst[:], inv_s["Wo"][:], None, op0=OP.mult)
        e2.tensor_scalar(wo8[:, 2 * chunk:2 * chunk + 2], wi8[:], 1, -1,
                         op0=OP.min, op1=OP.max)

    # ---------------- P3: projections --------------------------------------
    psA.close()
    psB = ExitStack()
    ppb = psB.enter_context(tc.tile_pool(name="ppb", bufs=1, space="PSUM"))

    q_sb = sb.tile([128, 2, T], F16)
    k_sb = sb.tile([128, 2, T], F16)
    for name, dst in (("Wq", q_sb), ("Wk", k_sb)):
        for p in range(2):
            for ch in range(NCH):
                tsl = slice(CH * ch, CH * (ch + 1))
                mm = ppb.tile([128, CH], F32, tag="mmq", bufs=2,
                              name=f"mm_{name}{p}{ch}")
                for ctp in range(NCP):
                    for si, src in enumerate((a8, b8)):
                        nc.tensor.matmul(
                            mm[:], w8[name][:, 2 * ctp:2 * ctp + 2, 128 * p:128 * (p + 1)],
                            src[:, 2 * ctp:2 * ctp + 2, tsl],
                            start=(ctp == 0 and si == 0),
                            stop=(ctp == NCP - 1 and si == 1), perf_mode=DR)
                raw = fr.tile([128, CH], F16, tag="raw", bufs=2,
                              name=f"raw_{name}{p}{ch}")
                nc.scalar.activation(raw[:], mm[:], ACTF.Copy)
                jq = ppb.tile([128, CH], F32, tag="jq", bufs=2,
                              name=f"jq_{name}{p}{ch}")
                nc.tensor.matmul(jq[:], jt[:], raw[:], start=True, stop=True)
                p1 = fr.tile([128, CH], F16, tag="p1", bufs=2,
                             name=f"p1_{name}{p}{ch}")
                nc.vector.tensor_tensor(p1[:], mm[:], t1s[:, tsl], op=OP.mult)
                p2t = fr.tile([128, CH], F16, tag="p2", bufs=2,
                              name=f"p2_{name}{p}{ch}")
                nc.gpsimd.tensor_tensor(p2t[:], jq[:], t2s[:, tsl], op=OP.mult)
                nc.vector.tensor_tensor(dst[:, p, tsl], p1[:], p2t[:], op=OP.add)

    v_sb = sb.tile([128, NTT, HPC, 65], F16)
    nc.gpsimd.memset(v_sb[:, :, :, 64:65], 1.0)
    for tt in range(NTT):
        mmv = ppb.tile([128, HD], F32, tag="mmv", bufs=3, name=f"mmv_{tt}")
        for ctp in range(NCP):
            for si, src in enumerate((a8, b8)):
                nc.tensor.matmul(
                    mmv[:], src[:, 2 * ctp:2 * ctp + 2, 128 * tt:128 * (tt + 1)],
                    w8["Wv"][:, 2 * ctp:2 * ctp + 2, :],
                    start=(ctp == 0 and si == 0),
                    stop=(ctp == NCP - 1 and si == 1), perf_mode=DR)
        nc.scalar.activation(v_sb[:, tt, :, 0:64],
                             mmv[:].rearrange("p (h d) -> p h d", h=HPC),
                             ACTF.Copy, scale=sv[:, tt])

    # ---------------- P4: attention (4 instances) ---------------------------
    expsc = sb.tile([128, 1], F32)
    nc.vector.tensor_tensor(expsc[:], swc["Wq"][:], swc["Wk"][:], op=OP.mult)
    nc.vector.tensor_scalar(expsc[:], expsc[:], 1.0 / np.sqrt(np.float64(D)), None,
                            op0=OP.mult)

    front.close()
    psB.close()
    expp = es.enter_context(tc.tile_pool(name="expp", bufs=1))
    tail = es.enter_context(tc.tile_pool(name="tail", bufs=1))
    psC = ExitStack()
    ppc = psC.enter_context(tc.tile_pool(name="ppc", bufs=1, space="PSUM"))

    y_sb = sb.tile([128, NTT, HPC, D], F16)

    def attention_pair(p):
        for e in range(2):
            h = 2 * p + e
            for jb in range(NQB):
                yaug = ppc.tile([65, QB], F32, tag="yaug", name=f"yaug{h}{jb}")
                nkt = (jb + 1) * (QB // KT)
                ngrp = nkt // KG
                for kgi, kg in enumerate(reversed(range(ngrp))):
                    sgrp = ppc.tile([128, KG * QB], F32, tag="sgrp", bufs=3,
                                    name=f"sgrp{h}{jb}{kg}")
                    for m in range(KG):
                        kt = kg * KG + m
                        nc.tensor.matmul(
                            sgrp[:, QB * m:QB * (m + 1)],
                            k_sb[64 * e:64 * (e + 1), p, 128 * kt:128 * (kt + 1)],
                            q_sb[64 * e:64 * (e + 1), p, QB * jb:QB * (jb + 1)],
                            start=True, stop=True, tile_position=(64 * e, 0))
                    egrp = expp.tile([128, KG * QB], F16, tag=f"egrp{e}", bufs=2,
                                     name=f"egrp{h}{jb}{kg}")
                    nc.scalar.activation(egrp[:], sgrp[:], ACTF.Exp, scale=expsc[:])
                    for m in range(KG):
                        kt = kg * KG + m
                        mbase = QB * jb - KT * kt
                        if mbase < 127:   # diagonal tile: causal mask needed
                            nc.gpsimd.affine_select(
                                out=egrp[:, QB * m:QB * (m + 1)],
                                in_=egrp[:, QB * m:QB * (m + 1)],
                                compare_op=OP.is_ge, fill=0.0,
                                base=mbase, pattern=[[1, QB]],
                                channel_multiplier=-1)
                    for m in range(KG):
                        kt = kg * KG + m
                        nc.tensor.matmul(yaug[:], v_sb[:, kt, h, :],
                                         egrp[:, QB * m:QB * (m + 1)],
                                         start=(kgi == 0 and m == 0),
                                         stop=(kgi == ngrp - 1 and m == KG - 1))
                yaug16 = expp.tile([65, QB], F16, tag=f"yaug16_{e}", bufs=2,
                                   name=f"yaug16{h}{jb}")
                nc.vector.tensor_copy(yaug16[:], yaug[:])
                for chn in range(QB // 128):
                    trp = ppc.tile([128, 65], F16, tag="trp", bufs=1,
                                   name=f"trp{h}{jb}{chn}")
                    nc.tensor.transpose(trp[:], yaug16[:, 128 * chn:128 * (chn + 1)],
                                        id16[0:65, 0:65])
                    rec = expp.tile([128, 1], F32, tag=f"rec{e}", bufs=2,
                                    name=f"rec{h}{jb}{chn}")
                    nc.vector.reciprocal(rec[:], trp[:, 64:65])
                    nc.vector.tensor_scalar(
                        y_sb[:, 4 * jb + chn, h, :], trp[:, 0:64],
                        rec[:], None, op0=OP.mult)

    def send_half(pair, a2a_in, a2a_out):
        hsl = slice(2 * pair, 2 * pair + 2)
        for d in range(NCORES):
            nc.sync.dma_start(
                a2a_in[d].rearrange("(p t h dd) -> p t h dd", p=128, t=2, h=2),
                y_sb[:, 2 * d:2 * d + 2, hsl, :])
        nc.gpsimd.collective_compute(
            "AllToAll", OP.bypass, replica_groups=[list(range(NCORES))],
            ins=[a2a_in.opt()], outs=[a2a_out.opt()])

    attention_pair(0)
    send_half(0, a2aA_in, a2aA_out)
    attention_pair(1)
    send_half(1, a2aB_in, a2aB_out)

    # ---------------- P5: arrivals + output projection ----------------------
    yfull = tail.tile([128, OTT, C], F16)
    for half, a2a_out in ((0, a2aA_out), (1, a2aB_out)):
        for s in range(NCORES):
            csl = slice(256 * (s % 4) + 128 * half, 256 * (s % 4) + 128 * (half + 1))
            nc.sync.dma_start(
                yfull[:, 2 * (s // 4):2 * (s // 4) + 2, csl],
                a2a_out[s].rearrange("(p t c) -> p t c", p=128, t=2))

    psC.close()
    psD = ExitStack()
    ppd = psD.enter_context(tc.tile_pool(name="ppd", bufs=1, space="PSUM"))

    mxy = tail.tile([128, OTT, 1], F32)
    for tt in range(OTT):
        nc.vector.tensor_reduce(mxy[:, tt], yfull[:, tt], axis=AX.X, op=OP.max,
                                apply_absolute_value=True)
    scy = tail.tile([128, OTT, 1], F32)
    nc.vector.tensor_scalar(scy[:], mxy[:], 1e-5, 1.0 / 127.0, op0=OP.max, op1=OP.mult)
    sty = tail.tile([128, OTT, 1], F32)
    nc.vector.reciprocal(sty[:], scy[:])
    osc = tail.tile([128, OTT, 1], F32)
    nc.vector.tensor_scalar(osc[:], scy[:], swc["Wo"][:], None, op0=OP.mult)

    ya8 = tail.tile([128, OTT, C], F8)
    yb8 = tail.tile([128, OTT, C], F8)
    for tt in range(OTT):
        yq8 = tail.tile([128, C], I8, tag="yq8", bufs=2, name=f"yq8_{tt}")
        nc.vector.tensor_scalar(yq8[:], yfull[:, tt], sty[:, tt], None, op0=OP.mult)
        nc.gpsimd.tensor_copy(ya8[:, tt], yq8[:])
        eng = nc.vector if tt % 2 == 0 else nc.gpsimd
        eng.tensor_tensor(yb8[:, tt], yq8[:], ya8[:, tt], op=OP.subtract)

    yaT = tail.tile([128, NCT, OT], F8)
    ybT = tail.tile([128, NCT, OT], F8)
    for src, dstT in ((ya8, yaT), (yb8, ybT)):
        for ct in range(NCT):
            tr8 = ppd.tile([128, OT], F8, tag="tr8", bufs=2,
                           name=f"tr8_{ct}_{dstT is ybT}")
            for tt in range(OTT):
                nc.tensor.transpose(tr8[:, 128 * tt:128 * (tt + 1)],
                                    src[:, tt, 128 * ct:128 * (ct + 1)], id8[:])
            eng = (nc.vector, nc.gpsimd, nc.scalar)[ct % 3]
            if eng is nc.scalar:
                eng.activation(dstT[:, ct], tr8[:], ACTF.Copy)
            else:
                eng.tensor_copy(dstT[:, ct], tr8[:])

    outR = io["out_slice"].rearrange("(tt p) c -> p tt c", p=128)
    for tt in range(OTT):
        pso = ppd.tile([128, C], F32, tag="pso", bufs=2, name=f"pso_{tt}")
        for ctp in range(NCP):
            for si, srcT in enumerate((yaT, ybT)):
                nc.tensor.matmul(
                    pso[:], srcT[:, 2 * ctp:2 * ctp + 2, 128 * tt:128 * (tt + 1)],
                    wo8[:, 2 * ctp:2 * ctp + 2, :],
                    start=(ctp == 0 and si == 0),
                    stop=(ctp == NCP - 1 and si == 1), perf_mode=DR)
        outt = tail.tile([128, C], F32, tag="outt", bufs=2, name=f"outt_{tt}")
        nc.scalar.activation(outt[:], pso[:], ACTF.Copy, scale=osc[:, tt])
        nc.sync.dma_start(outR[:, tt], outt[:])
    psD.close()
    es.close()


def kernel(x, Wq, Wk, Wv, Wo, _trace=False):
    x = np.ascontiguousarray(np.asarray(x, np.float32))
    if "nc" not in _CACHE:
        _CACHE["nc"] = build_program()
    nc = _CACHE["nc"]
    t1, t2 = _host_tables()
    jth = _host_jt()
    ws = {"Wq": np.asarray(Wq, np.float32), "Wk": np.asarray(Wk, np.float32),
          "Wv": np.asarray(Wv, np.float32), "Wo": np.asarray(Wo, np.float32)}
    wstack = np.stack([ws["Wq"], ws["Wk"], ws["Wv"], ws["Wo"]], axis=0)
    woT = np.ascontiguousarray(ws["Wo"].T)
    in_maps = []
    for c in range(NCORES):
        b, j = c // 4, c % 4
        in_maps.append({
            "xT16": np.ascontiguousarray(x[b].T.astype(np.float16)),
            "Wshards": np.ascontiguousarray(
                wstack[:, 128 * c:128 * (c + 1), :].transpose(1, 0, 2)),
            "WqTs": np.ascontiguousarray(ws["Wq"][HD * j:HD * (j + 1), :].T),
            "WkTs": np.ascontiguousarray(ws["Wk"][HD * j:HD * (j + 1), :].T),
            "WvTs": np.ascontiguousarray(ws["Wv"][HD * j:HD * (j + 1), :].T),
            "WoT": woT,
            "ropeT1": t1, "ropeT2": t2, "ropeJT": jth,
        })
    res = run_bass_kernel_spmd(nc, in_maps, list(range(NCORES)), trace=_trace)
    out = np.zeros((B, T, C), np.float32)
    for c in range(NCORES):
        o = np.asarray(res.results[c]["out_slice"])
        out[0, 256 * c:256 * (c + 1)] = o[0:256]
        out[1, 256 * c:256 * (c + 1)] = o[256:512]
    if _trace:
        return out, res
    return out


# revision 14
# speedup vs baseline: 1.2578x; 1.0568x over previous
"""Trainium2 Bass kernel for nn_CausalSelfAttention_52905407152466.

BitNet-style causal self-attention, distributed over 8 NeuronCores with an
instance-parallel (batch x head-group) sharding that needs NO collective
before attention:

  - core c owns batch b=c//4 and heads {4j..4j+3} with j=c%4.  It receives
    its batch's x (transposed, fp16) and the column slices of Wq/Wk/Wv for
    its heads, computes q,k,v for all 2048 tokens of its batch locally, and
    runs causal attention for its 4 heads.
  - the ternary weight scales (mean|W|) need the full matrices; each core
    reduces a 1/8 row shard of each W and a 128-byte AllGather combines the
    partial sums (fully overlapped with the x pipeline).
  - y reshards to token-sharded via FOUR quarter AllToAlls, one per head,
    each issued as soon as that head's attention finishes so only the last
    quarter's transfer is exposed.  Arriving quarters are transposed to
    channel-major during attention, so the tail is just quant + Wo matmuls.

Numerics: act_quant int8 values split EXACTLY into two fp8e4m3 operands
(a=fp8(v), b=v-a with |b|<=4; both exact), and ternary weights are exact in
fp8, so every projection runs as DoubleRow fp8 matmuls (2x fp16 throughput)
while reproducing the reference int8xternary products exactly (fp32 psum).
Per-token activation-quant scales are folded into the rope tables (q,k),
the exp scale (sw_q*sw_k/sqrt(D)), the v psum copy (sc*sw_v) and the output
copy (scy*sw_o).  Attention runs in fp16 with the ones-column-in-V
normalizer; diagonal score tiles are processed ragged (only the visible
columns are computed/exponentiated) with a single 128-wide affine_select
boundary mask per tile.
"""

import numpy as np

import concourse.bacc as bacc
import concourse.mybir as mybir
import concourse.tile as tile
from concourse.bass_utils import run_bass_kernel_spmd
from concourse.masks import make_identity

F32 = mybir.dt.float32
F16 = mybir.dt.float16
F8 = mybir.dt.float8e4
I8 = mybir.dt.int8
AX = mybir.AxisListType
OP = mybir.AluOpType
ACTF = mybir.ActivationFunctionType
DR = mybir.MatmulPerfMode.DoubleRow

NCORES = 8
B, T, C = 2, 2048, 1024
H, D = 16, 64
HPC = 4                     # heads per core
HD = HPC * D                # 256 projection channels per core
NCT = C // 128              # 8 channel tiles
NCP = NCT // 2              # 4 channel-tile pairs (DoubleRow)
NTT = T // 128              # 16 token tiles per batch
QB = 512                    # query block
NQB = T // QB               # 4
KT = 128                    # key tile
OT = 512                    # owned output tokens per core (256 per batch)
OTT = OT // 128             # 4
CH = 512                    # q/k projection token chunk
NCH = T // CH               # 4
QSZ = 128 * 2 * D           # a2a quarter slot elems: 128p x 2 tiles x 64
ROPE_BASE = 10000.0

_CACHE = {}


def _host_tables():
    pos = np.arange(T, dtype=np.float64)
    inv = 1.0 / (ROPE_BASE ** (np.arange(0, D, 2, dtype=np.float64) / D))
    ang = pos[None, :] * inv[:, None]              # [32, T]
    cos = np.cos(ang).astype(np.float32).astype(np.float16)
    sin = np.sin(ang).astype(np.float32).astype(np.float16)
    t1 = np.concatenate([cos, cos, cos, cos], axis=0)
    t2 = np.concatenate([sin, sin, sin, sin], axis=0)
    return t1.astype(np.float16), t2.astype(np.float16)


def _host_jt():
    i32 = np.eye(32, dtype=np.float16)
    z = np.zeros((32, 32), np.float16)
    j64 = np.block([[z, -i32], [i32, z]])     # J: Jq[0:32] = -q[32:64]; Jq[32:64] = q[0:32]
    jt = np.block([[j64.T, np.zeros((64, 64), np.float16)],
                   [np.zeros((64, 64), np.float16), j64.T]])
    return jt.astype(np.float16)


def build_program():
    nc = bacc.Bacc("TRN2", target_bir_lowering=False, debug=False,
                   num_devices=NCORES)
    io = {}

    def inp(name, shape, dtype=F32):
        io[name] = nc.declare_dram_parameter(name, list(shape), dtype, isOutput=False)
        return io[name]

    def outp(name, shape, dtype=F32):
        io[name] = nc.declare_dram_parameter(name, list(shape), dtype, isOutput=True)
        return io[name]

    inp("xT16", (C, T), F16)
    inp("Wshards", (128, 4, C))
    for n in ("Wq", "Wk", "Wv"):
        inp(n + "Ts", (C, HD))
    inp("WoT", (C, C))
    inp("ropeT1", (128, T), F16)
    inp("ropeT2", (128, T), F16)
    inp("ropeJT", (128, 128), F16)
    outp("out_slice", (OT, C))

    with tile.TileContext(nc) as tc:
        with tc.tile_pool(name="dram", bufs=1, space="DRAM") as dram:
            ag_in = dram.tile([1, 4], F32)
            ag_out = dram.tile([8, 4], F32)
            a2a = [(dram.tile([NCORES, QSZ], F16, name=f"a2a_in{q}"),
                    dram.tile([NCORES, QSZ], F16, name=f"a2a_out{q}"))
                   for q in range(HPC)]
            _build_body(nc, tc, io, ag_in, ag_out, a2a)
    nc.compile()
    return nc


def _build_body(nc, tc, io, ag_in, ag_out, a2a):
    from contextlib import ExitStack
    es = ExitStack()
    const = es.enter_context(tc.tile_pool(name="const", bufs=1))
    sb = es.enter_context(tc.tile_pool(name="sb", bufs=1))
    front = ExitStack()
    fr = front.enter_context(tc.tile_pool(name="fr", bufs=1))
    wl = front.enter_context(tc.tile_pool(name="wl", bufs=1))
    psA = ExitStack()
    ppa = psA.enter_context(tc.tile_pool(name="ppa", bufs=1, space="PSUM"))

    # ---------------- constants -------------------------------------------
    id16 = const.tile([128, 128], F16)
    make_identity(nc, id16[:])
    id8 = const.tile([128, 128], F8)
    make_identity(nc, id8[:])
    t1 = const.tile([128, T], F16)
    t2 = const.tile([128, T], F16)
    jt = const.tile([128, 128], F16)
    nc.scalar.dma_start(t1[:], io["ropeT1"][:])
    nc.scalar.dma_start(t2[:], io["ropeT2"][:])
    nc.scalar.dma_start(jt[:], io["ropeJT"][:])
    ones_col = const.tile([128, 1], F32)
    nc.gpsimd.memset(ones_col[:], 1.0)
    ones_row = const.tile([1, 128], F32)
    nc.gpsimd.memset(ones_row[:], 1.0)
    ones_row16 = const.tile([1, 128], F16)
    nc.gpsimd.memset(ones_row16[:], 1.0)

    # ---------------- P0: weight-scale shards + AllGather ------------------
    asums = fr.tile([128, 4], F32)
    for w in range(4):
        wsh = fr.tile([128, 1, C], F32, tag="wsh", bufs=2, name=f"wsh_{w}")
        nc.sync.dma_start(wsh[:], io["Wshards"][:, w:w + 1])
        nc.vector.tensor_reduce(asums[:, w:w + 1], wsh[:, 0], axis=AX.X, op=OP.add,
                                apply_absolute_value=True)
    part_ps = ppa.tile([1, 4], F32, tag="pp_small")
    nc.tensor.matmul(part_ps[:], ones_col[:], asums[:], start=True, stop=True)
    part_sb = fr.tile([1, 4], F32)
    nc.vector.tensor_copy(part_sb[:], part_ps[:])
    nc.scalar.dma_start(ag_in[:], part_sb[:])
    nc.gpsimd.collective_compute(
        "AllGather", OP.bypass, replica_groups=[list(range(NCORES))],
        ins=[ag_in.opt()], outs=[ag_out.opt()])
    gath = fr.tile([8, 4], F32)
    nc.scalar.dma_start(gath[:], ag_out[:])
    tot_ps = ppa.tile([1, 4], F32, tag="pp_small", name="tot_ps")
    nc.tensor.matmul(tot_ps[:], ones_col[0:8], gath[:], start=True, stop=True)
    sw_row = fr.tile([1, 4], F32)
    nc.vector.tensor_scalar(sw_row[:], tot_ps[:], 1.0 / (C * C), 1e-5,
                            op0=OP.mult, op1=OP.max)
    swc = {}
    inv_s = {}
    for wi, wn in enumerate(("Wq", "Wk", "Wv", "Wo")):
        swb_ps = ppa.tile([128, 1], F32, tag="pp_small2", name=f"swb_{wn}")
        nc.tensor.matmul(swb_ps[:], ones_row[:], sw_row[:, wi:wi + 1],
                         start=True, stop=True)
        c_ = sb.tile([128, 1], F32, name=f"swc_{wn}")
        nc.vector.tensor_copy(c_[:], swb_ps[:])
        swc[wn] = c_
        iv = sb.tile([128, 1], F32, name=f"invs_{wn}")
        nc.vector.reciprocal(iv[:], c_[:])
        inv_s[wn] = iv

    # ---------------- P1: x load + per-token absmax ------------------------
    xT = fr.tile([128, NCT, T], F16)
    xTr = io["xT16"].rearrange("(ct p) t -> p ct t", p=128)
    accA = fr.tile([128, T], F16)
    accB = fr.tile([128, T], F16)
    for ct in range(NCT):
        eng = nc.sync if ct % 2 == 0 else nc.scalar
        eng.dma_start(xT[:, ct], xTr[:, ct])
        acc, veng = ((accA, nc.vector), (accB, nc.gpsimd))[ct % 2]
        if ct < 2:
            veng.tensor_scalar(acc[:], xT[:, ct], 0.0, None, op0=OP.abs_max)
        else:
            veng.tensor_tensor(acc[:], acc[:], xT[:, ct], op=OP.abs_max)
    acc = accA
    nc.vector.tensor_tensor(acc[:], accA[:], accB[:], op=OP.abs_max)

    # ---------------- P2: weight slices, ternarize to fp8 ------------------
    w8 = {}
    for i, wn in enumerate(("Wq", "Wk", "Wv")):
        wst = wl.tile([128, NCT, HD], F32, tag="wst", bufs=2, name=f"wst_{wn}")
        nc.sync.dma_start(wst[:], io[wn + "Ts"].rearrange("(ct p) o -> p ct o", p=128))
        wi8 = fr.tile([128, NCT, HD], I8, tag="wi8", bufs=2, name=f"wi8_{wn}")
        e1, e2 = (nc.vector, nc.gpsimd) if i % 2 == 0 else (nc.gpsimd, nc.vector)
        e1.tensor_scalar(wi8[:], wst[:], inv_s[wn][:], None, op0=OP.mult)
        wf = sb.tile([128, NCT, HD], F8, name=f"w8_{wn}")
        e2.tensor_scalar(wf[:], wi8[:], 1, -1, op0=OP.min, op1=OP.max)
        w8[wn] = wf

    # ---------------- P1b: per-token scales + quantize ----------------------
    # per-token channel max is a partition (C-axis) reduction on gpsimd
    mx_row = fr.tile([1, T], F32)
    nc.gpsimd.tensor_reduce(mx_row[:], acc[:], axis=AX.C, op=OP.max)
    sc_row = mx_row   # in-place: mx_row becomes the dequant scale row
    nc.vector.tensor_scalar(sc_row[:], mx_row[:], 1e-5, 1.0 / 127.0,
                            op0=OP.max, op1=OP.mult)
    st_row = fr.tile([1, T], F32)
    nc.vector.reciprocal(st_row[:], sc_row[:])
    scrow16 = fr.tile([1, T], F16)
    nc.vector.tensor_copy(scrow16[:], sc_row[:])

    # broadcast sc (f16, dequant) then st (f32, grid-exact) along partitions
    scb_ps = ppa.tile([128, T], F32, tag="bigps")
    nc.tensor.matmul(scb_ps[:], ones_row16[:], scrow16[:], start=True, stop=True)
    scb16 = fr.tile([128, T], F16)
    nc.gpsimd.tensor_copy(scb16[:], scb_ps[:])
    t1s = sb.tile([128, T], F16)
    t2s = sb.tile([128, T], F16)
    nc.vector.tensor_tensor(t1s[:], t1[:], scb16[:], op=OP.mult)
    nc.gpsimd.tensor_tensor(t2s[:], t2[:], scb16[:], op=OP.mult)
    stb = ppa.tile([128, T], F32, tag="bigps", name="stb")  # psum through quant
    nc.tensor.matmul(stb[:], ones_row[:], st_row[:], start=True, stop=True)

    # token-major v-copy scale sv[p, tt] = sc(token 128*tt+p) * sw_v, via
    # transposing the (all-rows-identical) broadcast back per token tile
    sv = fr.tile([128, NTT, 1], F32)
    for tt in range(NTT):
        trs = ppa.tile([128, 128], F16, tag="trs", bufs=1, name=f"trs_{tt}")
        nc.tensor.transpose(trs[:], scb16[:, 128 * tt:128 * (tt + 1)], id16[:])
        nc.vector.tensor_scalar(sv[:, tt], trs[:, 0:1], swc["Wv"][:], None,
                                op0=OP.mult)

    # quantize: xq8 int8 grid == fp8 a + fp8 b exactly
    a8 = fr.tile([128, NCT, T], F8)
    b8 = fr.tile([128, NCT, T], F8)
    for ct in range(NCT):
        xq8 = fr.tile([128, T], I8, tag="xq8", bufs=2, name=f"xq8_{ct}")
        nc.vector.tensor_tensor(xq8[:], xT[:, ct], stb[:], op=OP.mult)
        nc.scalar.activation(a8[:, ct], xq8[:], ACTF.Copy)
        nc.gpsimd.tensor_tensor(b8[:, ct], xq8[:], a8[:, ct], op=OP.subtract)

    # ---------------- P3: projections --------------------------------------
    psA.close()
    psB = ExitStack()
    ppb = psB.enter_context(tc.tile_pool(name="ppb", bufs=1, space="PSUM"))

    q_sb = sb.tile([128, 2, T], F16)
    k_sb = sb.tile([128, 2, T], F16)
    for name, dst in (("Wq", q_sb), ("Wk", k_sb)):
        for p in range(2):
            for ch in range(NCH):
                tsl = slice(CH * ch, CH * (ch + 1))
                mm = ppb.tile([128, CH], F32, tag="mmq", bufs=2,
                              name=f"mm_{name}{p}{ch}")
                for ctp in range(NCP):
                    for si, src in enumerate((a8, b8)):
                        nc.tensor.matmul(
                            mm[:], w8[name][:, 2 * ctp:2 * ctp + 2, 128 * p:128 * (p + 1)],
                            src[:, 2 * ctp:2 * ctp + 2, tsl],
                            start=(ctp == 0 and si == 0),
                            stop=(ctp == NCP - 1 and si == 1), perf_mode=DR)
                raw = fr.tile([128, CH], F16, tag="raw", bufs=2,
                              name=f"raw_{name}{p}{ch}")
                nc.scalar.activation(raw[:], mm[:], ACTF.Copy)
                jq = ppb.tile([128, CH], F32, tag="jq", bufs=2,
                              name=f"jq_{name}{p}{ch}")
                nc.tensor.matmul(jq[:], jt[:], raw[:], start=True, stop=True)
                p1 = fr.tile([128, CH], F16, tag="p1", bufs=2,
                             name=f"p1_{name}{p}{ch}")
                nc.vector.tensor_tensor(p1[:], mm[:], t1s[:, tsl], op=OP.mult)
                p2t = fr.tile([128, CH], F16, tag="p2", bufs=2,
                              name=f"p2_{name}{p}{ch}")
                nc.gpsimd.tensor_tensor(p2t[:], jq[:], t2s[:, tsl], op=OP.mult)
                nc.vector.tensor_tensor(dst[:, p, tsl], p1[:], p2t[:], op=OP.add)

    v_sb = sb.tile([128, NTT, HPC, 65], F16)
    nc.gpsimd.memset(v_sb[:, :, :, 64:65], 1.0)
    for tt in range(NTT):
        mmv = ppb.tile([128, HD], F32, tag="mmv", bufs=3, name=f"mmv_{tt}")
        for ctp in range(NCP):
            for si, src in enumerate((a8, b8)):
                nc.tensor.matmul(
                    mmv[:], src[:, 2 * ctp:2 * ctp + 2, 128 * tt:128 * (tt + 1)],
                    w8["Wv"][:, 2 * ctp:2 * ctp + 2, :],
                    start=(ctp == 0 and si == 0),
                    stop=(ctp == NCP - 1 and si == 1), perf_mode=DR)
        nc.scalar.activation(v_sb[:, tt, :, 0:64],
                             mmv[:].rearrange("p (h d) -> p h d", h=HPC),
                             ACTF.Copy, scale=sv[:, tt])

    # ---------------- P4: attention + per-head resharding -------------------
    expsc = sb.tile([128, 1], F32)
    nc.vector.tensor_tensor(expsc[:], swc["Wq"][:], swc["Wk"][:], op=OP.mult)
    nc.vector.tensor_scalar(expsc[:], expsc[:], 1.0 / np.sqrt(np.float64(D)), None,
                            op0=OP.mult)
    swco = sb.tile([128, 1], F32)
    nc.vector.tensor_copy(swco[:], swc["Wo"][:])
    osiv = sb.tile([128, 1], F32)
    nc.vector.tensor_copy(osiv[:], inv_s["Wo"][:])

    front.close()
    psB.close()
    expp = es.enter_context(tc.tile_pool(name="expp", bufs=1))
    tail = es.enter_context(tc.tile_pool(name="tail", bufs=1))
    psC = ExitStack()
    ppc = psC.enter_context(tc.tile_pool(name="ppc", bufs=1, space="PSUM"))

    y_sb = sb.tile([128, NTT, HPC, D], F16)
    # channel-major resharded y (built during attention from quarter arrivals)
    ycT = tail.tile([128, NCT, OT], F16)
    accY = tail.tile([128, OT], F16)
    nc.gpsimd.memset(accY[:], 0.0)

    # Wo loads + ternarize overlap attention (DMA and DVE are mostly idle)
    wo8 = tail.tile([128, NCT, C], F8)
    woR = io["WoT"].rearrange("(ct p) o -> p ct o", p=128)
    for chunk in range(4):
        wst = tail.tile([128, 2, C], F32, tag="woc", bufs=2, name=f"woc_{chunk}")
        nc.sync.dma_start(wst[:], woR[:, 2 * chunk:2 * chunk + 2])
        wi8o = tail.tile([128, 2, C], I8, tag="woi", bufs=2, name=f"woi_{chunk}")
        nc.vector.tensor_scalar(wi8o[:], wst[:], osiv[:], None, op0=OP.mult)
        nc.vector.tensor_scalar(wo8[:, 2 * chunk:2 * chunk + 2], wi8o[:], 1, -1,
                                op0=OP.min, op1=OP.max)

    def attention_head(p, e):
        h = 2 * p + e
        for jb in range(NQB):
            yaug = ppc.tile([65, QB], F32, tag="yaug", name=f"yaug{h}{jb}")
            state = {"first": True, "kt": 0}

            def av(egrp, esl, csl, stop):
                nc.tensor.matmul(yaug[:, csl], v_sb[:, state["kt"], h, :],
                                 egrp[:, esl], start=state["first"], stop=stop,
                                 skip_group_check=True)
                state["first"] = False

            # fully-visible k-tile pairs
            for fg in range(2 * jb):
                sgrp = ppc.tile([128, 2 * QB], F32, tag="sgrp", bufs=2,
                                name=f"sgrp{h}{jb}{fg}")
                for m in range(2):
                    kt = fg * 2 + m
                    nc.tensor.matmul(
                        sgrp[:, QB * m:QB * (m + 1)],
                        k_sb[64 * e:64 * (e + 1), p, 128 * kt:128 * (kt + 1)],
                        q_sb[64 * e:64 * (e + 1), p, QB * jb:QB * (jb + 1)],
                        start=True, stop=True, tile_position=(64 * e, 0))
                egrp = expp.tile([128, 2 * QB], F16, tag=f"egrp{e}", bufs=2,
                                 name=f"egrp{h}{jb}{fg}")
                nc.scalar.activation(egrp[:], sgrp[:], ACTF.Exp, scale=expsc[:])
                for m in range(2):
                    state["kt"] = fg * 2 + m
                    av(egrp, slice(QB * m, QB * (m + 1)), slice(0, QB),
                       stop=False)
            # ragged diagonal tiles (kt = 4*jb+i), packed in two psum tiles
            for dpair in range(2):
                widths = [QB - KT * (2 * dpair), QB - KT * (2 * dpair + 1)]
                wtot = sum(widths)
                sgrp = ppc.tile([128, wtot], F32, tag="sgrp", bufs=2,
                                name=f"sgrpd{h}{jb}{dpair}")
                offs = [0, widths[0]]
                for ii in range(2):
                    i = 2 * dpair + ii
                    kt = 4 * jb + i
                    nc.tensor.matmul(
                        sgrp[:, offs[ii]:offs[ii] + widths[ii]],
                        k_sb[64 * e:64 * (e + 1), p, 128 * kt:128 * (kt + 1)],
                        q_sb[64 * e:64 * (e + 1), p,
                             QB * jb + KT * i:QB * (jb + 1)],
                        start=True, stop=True, tile_position=(64 * e, 0))
                egrp = expp.tile([128, wtot], F16, tag=f"egrp{e}", bufs=2,
                                 name=f"egrpd{h}{jb}{dpair}")
                nc.scalar.activation(egrp[:], sgrp[:], ACTF.Exp, scale=expsc[:])
                for ii in range(2):
                    # staircase mask on the first 128 columns of each slice
                    nc.gpsimd.affine_select(
                        out=egrp[:, offs[ii]:offs[ii] + KT],
                        in_=egrp[:, offs[ii]:offs[ii] + KT],
                        compare_op=OP.is_ge, fill=0.0,
                        base=0, pattern=[[1, KT]], channel_multiplier=-1)
                for ii in range(2):
                    i = 2 * dpair + ii
                    state["kt"] = 4 * jb + i
                    av(egrp, slice(offs[ii], offs[ii] + widths[ii]),
                       slice(KT * i, QB),
                       stop=(dpair == 1 and ii == 1))
            # epilogue: transpose, normalize by the ones-column Z
            yaug16 = expp.tile([65, QB], F16, tag=f"yaug16_{e}", bufs=2,
                               name=f"yaug16{h}{jb}")
            nc.vector.tensor_copy(yaug16[:], yaug[:])
            for chn in range(QB // 128):
                trp = ppc.tile([128, 512], F16, tag="trx", bufs=2,
                               name=f"trp{h}{jb}{chn}")
                nc.tensor.transpose(trp[:, 0:65], yaug16[:, 128 * chn:128 * (chn + 1)],
                                    id16[0:65, 0:65])
                rec = expp.tile([128, 1], F32, tag=f"rec{e}", bufs=2,
                                name=f"rec{h}{jb}{chn}")
                nc.vector.reciprocal(rec[:], trp[:, 64:65])
                nc.vector.tensor_scalar(
                    y_sb[:, 4 * jb + chn, h, :], trp[:, 0:64],
                    rec[:], None, op0=OP.mult)

    def send_quarter(h):
        a2a_in, a2a_out = a2a[h]
        for d in range(NCORES):
            nc.sync.dma_start(
                a2a_in[d].rearrange("(p t dd) -> p t dd", p=128, t=2),
                y_sb[:, 2 * d:2 * d + 2, h, :])
        nc.gpsimd.collective_compute(
            "AllToAll", OP.bypass, replica_groups=[list(range(NCORES))],
            ins=[a2a_in.opt()], outs=[a2a_out.opt()])

    def recv_quarter(h):
        """Unpack quarter h into channel-major ycT and fold into accY.

        Source s holds global head 4*(s%4)+h, i.e. channels
        [64*(4*(s%4)+h), +64) -> ct = 2*(s%4) + h//2, lanes [64*(h%2), +64).
        """
        _, a2a_out = a2a[h]
        lo = 64 * (h % 2)
        for s in range(NCORES):
            ct = 2 * (s % 4) + h // 2
            yarr = tail.tile([128, 2, D], F16, tag="yarr", bufs=4,
                             name=f"yarr{h}{s}")
            nc.sync.dma_start(
                yarr[:], a2a_out[s].rearrange("(p t dd) -> p t dd", p=128, t=2))
            try8 = ppc.tile([128, 512], F16, tag="trx", bufs=2,
                            name=f"try{h}{s}")
            for tl in range(2):
                nc.tensor.transpose(try8[lo:lo + 64, 128 * tl:128 * (tl + 1)],
                                    yarr[:, tl], id16[:])
            tt_loc = 2 * (s // 4)
            csl = slice(128 * tt_loc, 128 * (tt_loc + 2))
            eng = (nc.vector, nc.gpsimd)[s % 2]
            eng.tensor_copy(ycT[lo:lo + 64, ct, csl], try8[lo:lo + 64, 0:256])
            eng.tensor_tensor(accY[lo:lo + 64, csl], accY[lo:lo + 64, csl],
                              try8[lo:lo + 64, 0:256], op=OP.abs_max)

    for p in range(2):
        for e in range(2):
            attention_head(p, e)
            send_quarter(2 * p + e)
            if 2 * p + e > 0:
                recv_quarter(2 * p + e - 1)
    recv_quarter(HPC - 1)

    # ---------------- P5: output quant + projection -------------------------
    psC.close()
    psD = ExitStack()
    ppd = psD.enter_context(tc.tile_pool(name="ppd", bufs=1, space="PSUM"))

    mxy_row = tail.tile([1, OT], F32)
    nc.gpsimd.tensor_reduce(mxy_row[:], accY[:], axis=AX.C, op=OP.max)
    scy_row = mxy_row
    nc.vector.tensor_scalar(scy_row[:], mxy_row[:], 1e-5, 1.0 / 127.0,
                            op0=OP.max, op1=OP.mult)
    sty_row = tail.tile([1, OT], F32)
    nc.vector.reciprocal(sty_row[:], scy_row[:])
    scyrow16 = tail.tile([1, OT], F16)
    nc.vector.tensor_copy(scyrow16[:], scy_row[:])
    styb = ppd.tile([128, OT], F32, tag="styb")
    nc.tensor.matmul(styb[:], ones_row[:], sty_row[:], start=True, stop=True)
    scyb_ps = ppd.tile([128, OT], F32, tag="scyb")
    nc.tensor.matmul(scyb_ps[:], ones_row16[:], scyrow16[:], start=True, stop=True)
    scyb16 = tail.tile([128, OT], F16)
    nc.gpsimd.tensor_copy(scyb16[:], scyb_ps[:])
    osc = tail.tile([128, OTT, 1], F32)
    for tt in range(OTT):
        trso = ppd.tile([128, 128], F16, tag="trso", bufs=1, name=f"trso_{tt}")
        nc.tensor.transpose(trso[:], scyb16[:, 128 * tt:128 * (tt + 1)], id16[:])
        nc.vector.tensor_scalar(osc[:, tt], trso[:, 0:1], swco[:], None,
                                op0=OP.mult)

    ya8 = tail.tile([128, NCT, OT], F8)
    yb8 = tail.tile([128, NCT, OT], F8)
    for ct in range(NCT):
        yq8 = tail.tile([128, OT], I8, tag="yq8", bufs=2, name=f"yq8_{ct}")
        nc.vector.tensor_tensor(yq8[:], ycT[:, ct], styb[:], op=OP.mult)
        nc.scalar.activation(ya8[:, ct], yq8[:], ACTF.Copy)
        nc.gpsimd.tensor_tensor(yb8[:, ct], yq8[:], ya8[:, ct], op=OP.subtract)

    outR = io["out_slice"].rearrange("(tt p) c -> p tt c", p=128)
    for tt in range(OTT):
        pso = ppd.tile([128, C], F32, tag="pso", bufs=2, name=f"pso_{tt}")
        for ctp in range(NCP):
            for si, srcT in enumerate((ya8, yb8)):
                nc.tensor.matmul(
                    pso[:], srcT[:, 2 * ctp:2 * ctp + 2, 128 * tt:128 * (tt + 1)],
                    wo8[:, 2 * ctp:2 * ctp + 2, :],
                    start=(ctp == 0 and si == 0),
                    stop=(ctp == NCP - 1 and si == 1), perf_mode=DR)
        outt = tail.tile([128, C], F32, tag="outt", bufs=2, name=f"outt_{tt}")
        nc.scalar.activation(outt[:], pso[:], ACTF.Copy, scale=osc[:, tt])
        nc.sync.dma_start(outR[:, tt], outt[:])
    psD.close()
    es.close()


def kernel(x, Wq, Wk, Wv, Wo, _trace=False):
    x = np.ascontiguousarray(np.asarray(x, np.float32))
    if "nc" not in _CACHE:
        _CACHE["nc"] = build_program()
    nc = _CACHE["nc"]
    t1, t2 = _host_tables()
    jth = _host_jt()
    ws = {"Wq": np.asarray(Wq, np.float32), "Wk": np.asarray(Wk, np.float32),
          "Wv": np.asarray(Wv, np.float32), "Wo": np.asarray(Wo, np.float32)}
    wstack = np.stack([ws["Wq"], ws["Wk"], ws["Wv"], ws["Wo"]], axis=0)
    woT = np.ascontiguousarray(ws["Wo"].T)
    in_maps = []
    for c in range(NCORES):
        b, j = c // 4, c % 4
        in_maps.append({
            "xT16": np.ascontiguousarray(x[b].T.astype(np.float16)),
            "Wshards": np.ascontiguousarray(
                wstack[:, 128 * c:128 * (c + 1), :].transpose(1, 0, 2)),
            "WqTs": np.ascontiguousarray(ws["Wq"][HD * j:HD * (j + 1), :].T),
            "WkTs": np.ascontiguousarray(ws["Wk"][HD * j:HD * (j + 1), :].T),
            "WvTs": np.ascontiguousarray(ws["Wv"][HD * j:HD * (j + 1), :].T),
            "WoT": woT,
            "ropeT1": t1, "ropeT2": t2, "ropeJT": jth,
        })
    res = run_bass_kernel_spmd(nc, in_maps, list(range(NCORES)), trace=_trace)
    out = np.zeros((B, T, C), np.float32)
    for c in range(NCORES):
        o = np.asarray(res.results[c]["out_slice"])
        out[0, 256 * c:256 * (c + 1)] = o[0:256]
        out[1, 256 * c:256 * (c + 1)] = o[256:512]
    if _trace:
        return out, res
    return out


# revision 17
# speedup vs baseline: 1.3754x; 1.0935x over previous
"""Trainium2 Bass kernel for nn_CausalSelfAttention_52905407152466.

BitNet-style causal self-attention, distributed over 8 NeuronCores with an
instance-parallel (batch x head-group) sharding that needs NO collective
before attention:

  - core c owns batch b=c//4 and heads {4j..4j+3} with j=c%4.  It receives
    its batch's x (transposed, fp16) and the column slices of Wq/Wk/Wv for
    its heads, computes q,k,v for all 2048 tokens of its batch locally, and
    runs causal attention for its 4 heads.
  - the ternary weight scales (mean|W|) need the full matrices; each core
    reduces a 1/8 row shard of each W and a 128-byte AllGather combines the
    partial sums (fully overlapped with the x pipeline).
  - y reshards to token-sharded via FOUR quarter AllToAlls, one per head,
    each issued as soon as that head's attention finishes so only the last
    quarter's transfer is exposed.  Arriving quarters are transposed to
    channel-major during attention, so the tail is just quant + Wo matmuls.

Numerics: act_quant int8 values split EXACTLY into two fp8e4m3 operands
(a=fp8(v), b=v-a with |b|<=4; both exact), and ternary weights are exact in
fp8, so every projection runs as DoubleRow fp8 matmuls (2x fp16 throughput)
while reproducing the reference int8xternary products exactly (fp32 psum).
Per-token activation-quant scales are folded into the rope tables (q,k),
the exp scale (sw_q*sw_k/sqrt(D)), the v psum copy (sc*sw_v) and the output
copy (scy*sw_o).  Attention runs in fp16 with the ones-column-in-V
normalizer; diagonal score tiles are processed ragged (only the visible
columns are computed/exponentiated) with a single 128-wide affine_select
boundary mask per tile.
"""

import numpy as np

import concourse.bacc as bacc
import concourse.mybir as mybir
import concourse.tile as tile
from concourse.bass_utils import run_bass_kernel_spmd
from concourse.masks import make_identity

F32 = mybir.dt.float32
F16 = mybir.dt.float16
F8 = mybir.dt.float8e4
I8 = mybir.dt.int8
AX = mybir.AxisListType
OP = mybir.AluOpType
ACTF = mybir.ActivationFunctionType
DR = mybir.MatmulPerfMode.DoubleRow

NCORES = 8
B, T, C = 2, 2048, 1024
H, D = 16, 64
HPC = 4                     # heads per core
HD = HPC * D                # 256 projection channels per core
NCT = C // 128              # 8 channel tiles
NCP = NCT // 2              # 4 channel-tile pairs (DoubleRow)
NTT = T // 128              # 16 token tiles per batch
QB = 512                    # query block
NQB = T // QB               # 4
KT = 128                    # key tile
OT = 512                    # owned output tokens per core (256 per batch)
OTT = OT // 128             # 4
CH = 512                    # q/k projection token chunk
NCH = T // CH               # 4
QSZ = 128 * 2 * D           # a2a quarter slot elems: 128p x 2 tiles x 64
ROPE_BASE = 10000.0

_CACHE = {}


def _host_tables():
    pos = np.arange(T, dtype=np.float64)
    inv = 1.0 / (ROPE_BASE ** (np.arange(0, D, 2, dtype=np.float64) / D))
    ang = pos[None, :] * inv[:, None]              # [32, T]
    cos = np.cos(ang).astype(np.float32).astype(np.float16)
    sin = np.sin(ang).astype(np.float32).astype(np.float16)
    t1 = np.concatenate([cos, cos, cos, cos], axis=0)
    t2 = np.concatenate([sin, sin, sin, sin], axis=0)
    return t1.astype(np.float16), t2.astype(np.float16)


def _host_jt():
    i32 = np.eye(32, dtype=np.float16)
    z = np.zeros((32, 32), np.float16)
    j64 = np.block([[z, -i32], [i32, z]])     # J: Jq[0:32] = -q[32:64]; Jq[32:64] = q[0:32]
    jt = np.block([[j64.T, np.zeros((64, 64), np.float16)],
                   [np.zeros((64, 64), np.float16), j64.T]])
    return jt.astype(np.float16)


def build_program():
    nc = bacc.Bacc("TRN2", target_bir_lowering=False, debug=False,
                   num_devices=NCORES)
    io = {}

    def inp(name, shape, dtype=F32):
        io[name] = nc.declare_dram_parameter(name, list(shape), dtype, isOutput=False)
        return io[name]

    def outp(name, shape, dtype=F32):
        io[name] = nc.declare_dram_parameter(name, list(shape), dtype, isOutput=True)
        return io[name]

    inp("xT16", (C, T), F16)
    inp("Wshards", (128, 4, C))
    for n in ("Wq", "Wk", "Wv"):
        inp(n + "Ts", (C, HD))
    inp("WoT", (C, C))
    inp("ropeT1", (128, T), F16)
    inp("ropeT2", (128, T), F16)
    inp("ropeJT", (128, 128), F16)
    outp("out_slice", (OT, C))

    with tile.TileContext(nc) as tc:
        with tc.tile_pool(name="dram", bufs=1, space="DRAM") as dram:
            ag_in = dram.tile([1, 4], F32)
            ag_out = dram.tile([8, 4], F32)
            a2a = [(dram.tile([NCORES, 2 * QSZ], F16, name=f"a2a_in{q}"),
                    dram.tile([NCORES, 2 * QSZ], F16, name=f"a2a_out{q}"))
                   for q in range(2)]
            _build_body(nc, tc, io, ag_in, ag_out, a2a)
    nc.compile()
    return nc


def _build_body(nc, tc, io, ag_in, ag_out, a2a):
    from contextlib import ExitStack
    from itertools import zip_longest
    es = ExitStack()
    const = es.enter_context(tc.tile_pool(name="const", bufs=1))
    sb = es.enter_context(tc.tile_pool(name="sb", bufs=1))
    front = ExitStack()
    fr = front.enter_context(tc.tile_pool(name="fr", bufs=1))
    wl = front.enter_context(tc.tile_pool(name="wl", bufs=1))
    psA = ExitStack()
    ppa = psA.enter_context(tc.tile_pool(name="ppa", bufs=1, space="PSUM"))

    # ---------------- constants -------------------------------------------
    id16 = const.tile([128, 128], F16)
    make_identity(nc, id16[:])
    id8 = const.tile([128, 128], F8)
    make_identity(nc, id8[:])
    jt = const.tile([128, 128], F16)
    ones_col = const.tile([128, 1], F32)
    nc.gpsimd.memset(ones_col[:], 1.0)
    ones_row = const.tile([1, 128], F32)
    nc.gpsimd.memset(ones_row[:], 1.0)
    ones_row16 = const.tile([1, 128], F16)
    nc.gpsimd.memset(ones_row16[:], 1.0)

    # ---------------- P1: x load + per-token absmax -------------------------
    # x DMAs lead the queues; weight-scale shards stream via the DVE queue so
    # the AllGather can fire ~12us in, overlapped with the absmax pass.
    xT = fr.tile([128, NCT, T], F16)
    xTr = io["xT16"].rearrange("(ct p) t -> p ct t", p=128)
    accA = fr.tile([128, T], F16)
    accB = fr.tile([128, T], F16)
    asums = fr.tile([128, 4], F32)
    for ct in range(NCT):
        if ct % 2 == 0:
            nc.sync.dma_start(xT[:, ct], xTr[:, ct])
        else:
            w = ct // 2
            wsh = fr.tile([128, 1, C], F32, tag="wsh", bufs=2, name=f"wsh_{w}")
            nc.scalar.dma_start(wsh[:], io["Wshards"][:, w:w + 1])
            nc.scalar.dma_start(xT[:, ct], xTr[:, ct])
            nc.vector.tensor_reduce(asums[:, w:w + 1], wsh[:, 0], axis=AX.X,
                                    op=OP.add, apply_absolute_value=True)
    part_ps = ppa.tile([1, 4], F32, tag="pp_small")
    nc.tensor.matmul(part_ps[:], ones_col[:], asums[:], start=True, stop=True)
    part_sb = fr.tile([1, 4], F32)
    nc.vector.tensor_copy(part_sb[:], part_ps[:])
    nc.scalar.dma_start(ag_in[:], part_sb[:])
    for ct in range(NCT):
        acc, veng = ((accA, nc.vector), (accB, nc.gpsimd))[ct % 2]
        if ct < 2:
            veng.tensor_scalar(acc[:], xT[:, ct], 0.0, None, op0=OP.abs_max)
        else:
            veng.tensor_tensor(acc[:], acc[:], xT[:, ct], op=OP.abs_max)
    acc = accA
    nc.vector.tensor_tensor(acc[:], accA[:], accB[:], op=OP.abs_max)
    # rope tables / J arrive behind x on the Act queue (needed ~20us in)
    t1 = const.tile([128, T], F16)
    t2 = const.tile([128, T], F16)
    nc.scalar.dma_start(t1[:], io["ropeT1"][:])
    nc.scalar.dma_start(t2[:], io["ropeT2"][:])
    nc.scalar.dma_start(jt[:], io["ropeJT"][:])
    # q/k/v weight slices queue behind x on the sync queue
    wsts = {}
    for wn in ("Wq", "Wk", "Wv"):
        wst = wl.tile([128, NCT, HD], F32, tag="wst", bufs=3, name=f"wst_{wn}")
        nc.sync.dma_start(wst[:], io[wn + "Ts"].rearrange("(ct p) o -> p ct o", p=128))
        wsts[wn] = wst

    # per-token channel max is a partition (C-axis) reduction on gpsimd
    mx_row = fr.tile([1, T], F32)
    nc.gpsimd.tensor_reduce(mx_row[:], acc[:], axis=AX.C, op=OP.max)
    # AllGather of the four weight |.| partial sums (fires ~12us, overlapped)
    nc.gpsimd.collective_compute(
        "AllGather", OP.bypass, replica_groups=[list(range(NCORES))],
        ins=[ag_in.opt()], outs=[ag_out.opt()])
    gath = fr.tile([8, 4], F32)
    nc.scalar.dma_start(gath[:], ag_out[:])

    # ---------------- P1b: per-token scales ---------------------------------
    sc_row = mx_row   # in-place: mx_row becomes the dequant scale row
    nc.vector.tensor_scalar(sc_row[:], mx_row[:], 1e-5, 1.0 / 127.0,
                            op0=OP.max, op1=OP.mult)
    st_row = fr.tile([1, T], F32)
    nc.vector.reciprocal(st_row[:], sc_row[:])
    scrow16 = fr.tile([1, T], F16)
    nc.vector.tensor_copy(scrow16[:], sc_row[:])

    # broadcast sc (f16, dequant) then st (f32, grid-exact) along partitions
    scb_ps = ppa.tile([128, T], F32, tag="bigps")
    nc.tensor.matmul(scb_ps[:], ones_row16[:], scrow16[:], start=True, stop=True)
    scb16 = fr.tile([128, T], F16)
    nc.gpsimd.tensor_copy(scb16[:], scb_ps[:])
    t1s = sb.tile([128, T], F16)
    t2s = sb.tile([128, T], F16)
    nc.vector.tensor_tensor(t1s[:], t1[:], scb16[:], op=OP.mult)
    nc.gpsimd.tensor_tensor(t2s[:], t2[:], scb16[:], op=OP.mult)
    stb = ppa.tile([128, T], F32, tag="bigps", name="stb")  # psum through quant
    nc.tensor.matmul(stb[:], ones_row[:], st_row[:], start=True, stop=True)

    # token-major v-copy scale sv[p, tt] = sc(token 128*tt+p) * sw_v, via
    # transposing the (all-rows-identical) broadcast back per token tile
    swc = {}
    inv_s = {}
    tot_ps = ppa.tile([1, 4], F32, tag="pp_small", name="tot_ps")
    nc.tensor.matmul(tot_ps[:], ones_col[0:8], gath[:], start=True, stop=True)
    sw_row = fr.tile([1, 4], F32)
    nc.vector.tensor_scalar(sw_row[:], tot_ps[:], 1.0 / (C * C), 1e-5,
                            op0=OP.mult, op1=OP.max)
    for wi, wn in enumerate(("Wq", "Wk", "Wv", "Wo")):
        swb_ps = ppa.tile([128, 1], F32, tag="pp_small2", name=f"swb_{wn}")
        nc.tensor.matmul(swb_ps[:], ones_row[:], sw_row[:, wi:wi + 1],
                         start=True, stop=True)
        c_ = sb.tile([128, 1], F32, name=f"swc_{wn}")
        nc.vector.tensor_copy(c_[:], swb_ps[:])
        swc[wn] = c_
        iv = sb.tile([128, 1], F32, name=f"invs_{wn}")
        nc.vector.reciprocal(iv[:], c_[:])
        inv_s[wn] = iv
    sv = fr.tile([128, NTT, 1], F32)
    for tt in range(NTT):
        trs = ppa.tile([128, 128], F16, tag="trs", bufs=1, name=f"trs_{tt}")
        nc.tensor.transpose(trs[:], scb16[:, 128 * tt:128 * (tt + 1)], id16[:])
        nc.vector.tensor_scalar(sv[:, tt], trs[:, 0:1], swc["Wv"][:], None,
                                op0=OP.mult)

    # quantize: xq8 int8 grid == fp8 a + fp8 b exactly
    a8 = fr.tile([128, NCT, T], F8)
    b8 = fr.tile([128, NCT, T], F8)
    for ct in range(NCT):
        xq8 = fr.tile([128, T], I8, tag="xq8", bufs=2, name=f"xq8_{ct}")
        nc.vector.tensor_tensor(xq8[:], xT[:, ct], stb[:], op=OP.mult)
        nc.scalar.activation(a8[:, ct], xq8[:], ACTF.Copy)
        nc.gpsimd.tensor_tensor(b8[:, ct], xq8[:], a8[:, ct], op=OP.subtract)

    # ---------------- P2: ternarize weight slices to fp8 --------------------
    w8 = {}
    for i, wn in enumerate(("Wq", "Wk", "Wv")):
        wi8 = fr.tile([128, NCT, HD], I8, tag="wi8", bufs=2, name=f"wi8_{wn}")
        e1, e2 = (nc.vector, nc.gpsimd) if i % 2 == 0 else (nc.gpsimd, nc.vector)
        e1.tensor_scalar(wi8[:], wsts[wn][:], inv_s[wn][:], None, op0=OP.mult)
        wf = sb.tile([128, NCT, HD], F8, name=f"w8_{wn}")
        e2.tensor_scalar(wf[:], wi8[:], 1, -1, op0=OP.min, op1=OP.max)
        w8[wn] = wf

    # ---------------- P3: projections --------------------------------------
    psA.close()
    psB = ExitStack()
    ppb = psB.enter_context(tc.tile_pool(name="ppb", bufs=1, space="PSUM"))

    q_sb = sb.tile([128, 2, T], F16)
    k_sb = sb.tile([128, 2, T], F16)
    for name, dst in (("Wq", q_sb), ("Wk", k_sb)):
        for p in range(2):
            for ch in range(NCH):
                tsl = slice(CH * ch, CH * (ch + 1))
                mm = ppb.tile([128, CH], F32, tag="mmq", bufs=2,
                              name=f"mm_{name}{p}{ch}")
                for ctp in range(NCP):
                    for si, src in enumerate((a8, b8)):
                        nc.tensor.matmul(
                            mm[:], w8[name][:, 2 * ctp:2 * ctp + 2, 128 * p:128 * (p + 1)],
                            src[:, 2 * ctp:2 * ctp + 2, tsl],
                            start=(ctp == 0 and si == 0),
                            stop=(ctp == NCP - 1 and si == 1), perf_mode=DR)
                raw = fr.tile([128, CH], F16, tag="raw", bufs=2,
                              name=f"raw_{name}{p}{ch}")
                nc.scalar.activation(raw[:], mm[:], ACTF.Copy)
                jq = ppb.tile([128, CH], F32, tag="jq", bufs=2,
                              name=f"jq_{name}{p}{ch}")
                nc.tensor.matmul(jq[:], jt[:], raw[:], start=True, stop=True)
                p1 = fr.tile([128, CH], F16, tag="p1", bufs=2,
                             name=f"p1_{name}{p}{ch}")
                nc.vector.tensor_tensor(p1[:], mm[:], t1s[:, tsl], op=OP.mult)
                p2t = fr.tile([128, CH], F16, tag="p2", bufs=2,
                              name=f"p2_{name}{p}{ch}")
                nc.gpsimd.tensor_tensor(p2t[:], jq[:], t2s[:, tsl], op=OP.mult)
                nc.vector.tensor_tensor(dst[:, p, tsl], p1[:], p2t[:], op=OP.add)

    v_sb = sb.tile([128, NTT, HPC, 65], F16)
    nc.gpsimd.memset(v_sb[:, :, :, 64:65], 1.0)
    for tt in range(NTT):
        mmv = ppb.tile([128, HD], F32, tag="mmv", bufs=3, name=f"mmv_{tt}")
        for ctp in range(NCP):
            for si, src in enumerate((a8, b8)):
                nc.tensor.matmul(
                    mmv[:], src[:, 2 * ctp:2 * ctp + 2, 128 * tt:128 * (tt + 1)],
                    w8["Wv"][:, 2 * ctp:2 * ctp + 2, :],
                    start=(ctp == 0 and si == 0),
                    stop=(ctp == NCP - 1 and si == 1), perf_mode=DR)
        nc.scalar.activation(v_sb[:, tt, :, 0:64],
                             mmv[:].rearrange("p (h d) -> p h d", h=HPC),
                             ACTF.Copy, scale=sv[:, tt])

    # ---------------- P4: attention + per-pair resharding -------------------
    expsc = sb.tile([128, 1], F32)
    nc.vector.tensor_tensor(expsc[:], swc["Wq"][:], swc["Wk"][:], op=OP.mult)
    nc.vector.tensor_scalar(expsc[:], expsc[:], 1.0 / np.sqrt(np.float64(D)), None,
                            op0=OP.mult)
    swco = sb.tile([128, 1], F32)
    nc.vector.tensor_copy(swco[:], swc["Wo"][:])
    osiv = sb.tile([128, 1], F32)
    nc.vector.tensor_copy(osiv[:], inv_s["Wo"][:])

    front.close()
    psB.close()
    expp = es.enter_context(tc.tile_pool(name="expp", bufs=1))
    tail = es.enter_context(tc.tile_pool(name="tail", bufs=1))
    psC = ExitStack()
    ppc = psC.enter_context(tc.tile_pool(name="ppc", bufs=1, space="PSUM"))

    y_sb = sb.tile([128, NTT, HPC, D], F16)
    # channel-major resharded y (built during attention from half arrivals)
    ycT = tail.tile([128, NCT, OT], F16)
    accY = tail.tile([128, OT], F16)
    nc.gpsimd.memset(accY[:], 0.0)

    # Wo loads + ternarize overlap attention (DMA and DVE are mostly idle)
    wo8 = tail.tile([128, NCT, C], F8)
    woR = io["WoT"].rearrange("(ct p) o -> p ct o", p=128)
    for chunk in range(4):
        wst = tail.tile([128, 2, C], F32, tag="woc", bufs=2, name=f"woc_{chunk}")
        nc.sync.dma_start(wst[:], woR[:, 2 * chunk:2 * chunk + 2])
        wi8o = tail.tile([128, 2, C], I8, tag="woi", bufs=2, name=f"woi_{chunk}")
        nc.vector.tensor_scalar(wi8o[:], wst[:], osiv[:], None, op0=OP.mult)
        nc.vector.tensor_scalar(wo8[:, 2 * chunk:2 * chunk + 2], wi8o[:], 1, -1,
                                op0=OP.min, op1=OP.max)

    def head_thunks(p, e):
        """List of emission thunks for head (p,e): one per score group plus
        one per epilogue block; interleaving two heads' thunks keeps the
        exp pipe and PE continuously fed."""
        h = 2 * p + e
        thunks = []
        state = {"first": True, "kt": 0, "yaug": None}

        def mk_yaug(jb):
            def f():
                state["yaug"] = ppc.tile([65, QB], F32, tag=f"yaug{e}",
                                         name=f"yaug{h}{jb}")
                state["first"] = True
            return f

        def av(egrp, esl, csl, stop):
            nc.tensor.matmul(state["yaug"][:, csl], v_sb[:, state["kt"], h, :],
                             egrp[:, esl], start=state["first"], stop=stop,
                             skip_group_check=True)
            state["first"] = False

        def mk_full(jb, fg):
            def f():
                sgrp = ppc.tile([128, 2 * QB], F32, tag="sgrp", bufs=2,
                                name=f"sgrp{h}{jb}{fg}")
                for m in range(2):
                    kt = fg * 2 + m
                    nc.tensor.matmul(
                        sgrp[:, QB * m:QB * (m + 1)],
                        k_sb[64 * e:64 * (e + 1), p, 128 * kt:128 * (kt + 1)],
                        q_sb[64 * e:64 * (e + 1), p, QB * jb:QB * (jb + 1)],
                        start=True, stop=True, tile_position=(64 * e, 0))
                egrp = expp.tile([128, 2 * QB], F16, tag=f"egrp{e}", bufs=2,
                                 name=f"egrp{h}{jb}{fg}")
                nc.scalar.activation(egrp[:], sgrp[:], ACTF.Exp, scale=expsc[:])
                for m in range(2):
                    state["kt"] = fg * 2 + m
                    av(egrp, slice(QB * m, QB * (m + 1)), slice(0, QB),
                       stop=False)
            return f

        def mk_diag(jb, dpair):
            def f():
                widths = [QB - KT * (2 * dpair), QB - KT * (2 * dpair + 1)]
                wtot = sum(widths)
                offs = [0, widths[0]]
                sgrp = ppc.tile([128, wtot], F32, tag="sgrp", bufs=2,
                                name=f"sgrpd{h}{jb}{dpair}")
                for ii in range(2):
                    i = 2 * dpair + ii
                    kt = 4 * jb + i
                    nc.tensor.matmul(
                        sgrp[:, offs[ii]:offs[ii] + widths[ii]],
                        k_sb[64 * e:64 * (e + 1), p, 128 * kt:128 * (kt + 1)],
                        q_sb[64 * e:64 * (e + 1), p,
                             QB * jb + KT * i:QB * (jb + 1)],
                        start=True, stop=True, tile_position=(64 * e, 0))
                egrp = expp.tile([128, wtot], F16, tag=f"egrp{e}", bufs=2,
                                 name=f"egrpd{h}{jb}{dpair}")
                nc.scalar.activation(egrp[:], sgrp[:], ACTF.Exp, scale=expsc[:])
                for ii in range(2):
                    # staircase mask on the first 128 columns of each slice
                    nc.gpsimd.affine_select(
                        out=egrp[:, offs[ii]:offs[ii] + KT],
                        in_=egrp[:, offs[ii]:offs[ii] + KT],
                        compare_op=OP.is_ge, fill=0.0,
                        base=0, pattern=[[1, KT]], channel_multiplier=-1)
                for ii in range(2):
                    i = 2 * dpair + ii
                    state["kt"] = 4 * jb + i
                    av(egrp, slice(offs[ii], offs[ii] + widths[ii]),
                       slice(KT * i, QB),
                       stop=(dpair == 1 and ii == 1))
            return f

        def mk_epi(jb):
            def f():
                yaug16 = expp.tile([65, QB], F16, tag=f"yaug16_{e}", bufs=2,
                                   name=f"yaug16{h}{jb}")
                nc.vector.tensor_copy(yaug16[:], state["yaug"][:])
                for chn in range(QB // 128):
                    trp = ppc.tile([128, 512], F16, tag="trx", bufs=2,
                                   name=f"trp{h}{jb}{chn}")
                    nc.tensor.transpose(trp[:, 0:65],
                                        yaug16[:, 128 * chn:128 * (chn + 1)],
                                        id16[0:65, 0:65])
                    rec = expp.tile([128, 1], F32, tag=f"rec{e}", bufs=2,
                                    name=f"rec{h}{jb}{chn}")
                    nc.vector.reciprocal(rec[:], trp[:, 64:65])
                    nc.vector.tensor_scalar(
                        y_sb[:, 4 * jb + chn, h, :], trp[:, 0:64],
                        rec[:], None, op0=OP.mult)
            return f

        for jb in range(NQB):
            thunks.append(mk_yaug(jb))
            for fg in range(2 * jb):
                thunks.append(mk_full(jb, fg))
            for dpair in range(2):
                thunks.append(mk_diag(jb, dpair))
            thunks.append(mk_epi(jb))
        return thunks

    def attention_pair(p):
        for a, b in zip_longest(head_thunks(p, 0), head_thunks(p, 1)):
            if a is not None:
                a()
            if b is not None:
                b()

    def send_half(ph):
        a2a_in, a2a_out = a2a[ph]
        for d in range(NCORES):
            nc.sync.dma_start(
                a2a_in[d].rearrange("(p t h dd) -> p t h dd", p=128, t=2, h=2),
                y_sb[:, 2 * d:2 * d + 2, 2 * ph:2 * ph + 2, :])
        nc.gpsimd.collective_compute(
            "AllToAll", OP.bypass, replica_groups=[list(range(NCORES))],
            ins=[a2a_in.opt()], outs=[a2a_out.opt()])

    def recv_half(ph):
        """Unpack half ph into channel-major ycT and fold into accY.

        Source s holds global heads {4*(s%4)+2*ph, +1}, i.e. channels
        [256*(s%4)+128*ph, +128) -> ct = 2*(s%4) + ph, lanes 0..127.
        """
        _, a2a_out = a2a[ph]
        for s in range(NCORES):
            ct = 2 * (s % 4) + ph
            yarr = tail.tile([128, 2, 2, D], F16, tag="yarr", bufs=4,
                             name=f"yarr{ph}{s}")
            nc.sync.dma_start(
                yarr[:],
                a2a_out[s].rearrange("(p t h dd) -> p t h dd", p=128, t=2, h=2))
            try8 = ppc.tile([128, 512], F16, tag="trx", bufs=2,
                            name=f"try{ph}{s}")
            for tl in range(2):
                for hh in range(2):
                    nc.tensor.transpose(
                        try8[64 * hh:64 * (hh + 1), 128 * tl:128 * (tl + 1)],
                        yarr[:, tl, hh], id16[:])
            tt_loc = 2 * (s // 4)
            csl = slice(128 * tt_loc, 128 * (tt_loc + 2))
            eng = (nc.vector, nc.gpsimd)[s % 2]
            eng.tensor_copy(ycT[:, ct, csl], try8[:, 0:256])
            eng.tensor_tensor(accY[:, csl], accY[:, csl], try8[:, 0:256],
                              op=OP.abs_max)

    attention_pair(0)
    send_half(0)
    attention_pair(1)
    send_half(1)
    recv_half(0)
    recv_half(1)

    # ---------------- P5: output quant + projection -------------------------
    psC.close()
    psD = ExitStack()
    ppd = psD.enter_context(tc.tile_pool(name="ppd", bufs=1, space="PSUM"))

    mxy_row = tail.tile([1, OT], F32)
    nc.gpsimd.tensor_reduce(mxy_row[:], accY[:], axis=AX.C, op=OP.max)
    scy_row = mxy_row
    nc.vector.tensor_scalar(scy_row[:], mxy_row[:], 1e-5, 1.0 / 127.0,
                            op0=OP.max, op1=OP.mult)
    sty_row = tail.tile([1, OT], F32)
    nc.vector.reciprocal(sty_row[:], scy_row[:])
    scyrow16 = tail.tile([1, OT], F16)
    nc.vector.tensor_copy(scyrow16[:], scy_row[:])
    styb = ppd.tile([128, OT], F32, tag="styb")
    nc.tensor.matmul(styb[:], ones_row[:], sty_row[:], start=True, stop=True)
    scyb_ps = ppd.tile([128, OT], F32, tag="scyb")
    nc.tensor.matmul(scyb_ps[:], ones_row16[:], scyrow16[:], start=True, stop=True)
    scyb16 = tail.tile([128, OT], F16)
    nc.gpsimd.tensor_copy(scyb16[:], scyb_ps[:])
    osc = tail.tile([128, OTT, 1], F32)
    for tt in range(OTT):
        trso = ppd.tile([128, 128], F16, tag="trso", bufs=1, name=f"trso_{tt}")
        nc.tensor.transpose(trso[:], scyb16[:, 128 * tt:128 * (tt + 1)], id16[:])
        nc.vector.tensor_scalar(osc[:, tt], trso[:, 0:1], swco[:], None,
                                op0=OP.mult)

    ya8 = tail.tile([128, NCT, OT], F8)
    yb8 = tail.tile([128, NCT, OT], F8)
    for ct in range(NCT):
        yq8 = tail.tile([128, OT], I8, tag="yq8", bufs=2, name=f"yq8_{ct}")
        nc.vector.tensor_tensor(yq8[:], ycT[:, ct], styb[:], op=OP.mult)
        nc.scalar.activation(ya8[:, ct], yq8[:], ACTF.Copy)
        nc.gpsimd.tensor_tensor(yb8[:, ct], yq8[:], ya8[:, ct], op=OP.subtract)

    outR = io["out_slice"].rearrange("(tt p) c -> p tt c", p=128)
    for tt in range(OTT):
        pso = ppd.tile([128, C], F32, tag="pso", bufs=2, name=f"pso_{tt}")
        for ctp in range(NCP):
            for si, srcT in enumerate((ya8, yb8)):
                nc.tensor.matmul(
                    pso[:], srcT[:, 2 * ctp:2 * ctp + 2, 128 * tt:128 * (tt + 1)],
                    wo8[:, 2 * ctp:2 * ctp + 2, :],
                    start=(ctp == 0 and si == 0),
                    stop=(ctp == NCP - 1 and si == 1), perf_mode=DR)
        outt = tail.tile([128, C], F32, tag="outt", bufs=2, name=f"outt_{tt}")
        nc.scalar.activation(outt[:], pso[:], ACTF.Copy, scale=osc[:, tt])
        nc.sync.dma_start(outR[:, tt], outt[:])
    psD.close()
    es.close()


def kernel(x, Wq, Wk, Wv, Wo, _trace=False):
    x = np.ascontiguousarray(np.asarray(x, np.float32))
    if "nc" not in _CACHE:
        _CACHE["nc"] = build_program()
    nc = _CACHE["nc"]
    t1, t2 = _host_tables()
    jth = _host_jt()
    ws = {"Wq": np.asarray(Wq, np.float32), "Wk": np.asarray(Wk, np.float32),
          "Wv": np.asarray(Wv, np.float32), "Wo": np.asarray(Wo, np.float32)}
    wstack = np.stack([ws["Wq"], ws["Wk"], ws["Wv"], ws["Wo"]], axis=0)
    woT = np.ascontiguousarray(ws["Wo"].T)
    in_maps = []
    for c in range(NCORES):
        b, j = c // 4, c % 4
        in_maps.append({
            "xT16": np.ascontiguousarray(x[b].T.astype(np.float16)),
            "Wshards": np.ascontiguousarray(
                wstack[:, 128 * c:128 * (c + 1), :].transpose(1, 0, 2)),
            "WqTs": np.ascontiguousarray(ws["Wq"][HD * j:HD * (j + 1), :].T),
            "WkTs": np.ascontiguousarray(ws["Wk"][HD * j:HD * (j + 1), :].T),
            "WvTs": np.ascontiguousarray(ws["Wv"][HD * j:HD * (j + 1), :].T),
            "WoT": woT,
            "ropeT1": t1, "ropeT2": t2, "ropeJT": jth,
        })
    res = run_bass_kernel_spmd(nc, in_maps, list(range(NCORES)), trace=_trace)
    out = np.zeros((B, T, C), np.float32)
    for c in range(NCORES):
        o = np.asarray(res.results[c]["out_slice"])
        out[0, 256 * c:256 * (c + 1)] = o[0:256]
        out[1, 256 * c:256 * (c + 1)] = o[256:512]
    if _trace:
        return out, res
    return out
